# revision 8
# baseline (speedup 1.0000x reference)
"""AttentionBlock3D (GroupNorm + single-head attention over 4096 tokens + residual)
on 8 Trainium2 NeuronCores.

Sharding: core c handles sample b=c//2 and query half h=c%2 (2048 queries).
The host rotates each sample's token axis so that each core's queries are
columns 0..2048 of its x_kv input (attention is permutation-invariant over
keys, and groupnorm stats are permutation-invariant), letting a single SPMD
program serve all 8 cores with no dynamic offsets.

Device-side math per core (all fp32):
  h = groupnorm(x)                       (bn_stats + indicator-matmul group reduce)
  q = Wq h + bq  [256,2048]              (host passes qkv_w^T; no on-chip transposes)
  k = Wk h + bk  [256,4096]
  vT = (Wv h)^T  [4096,256]              (computed directly in transposed layout)
  S^T tiles  [128 keys, 512 queries] = k_chunk^T-free matmuls
  es = exp(S^T * C^-0.5)                 (no max subtraction: |scores*scale| <~ 8)
  out2[c,q] = sum_m vT[m,c] es[m,q]      (v chunks stationary, es streams)
  den[q]    = sum_m es[m,q]              (ones-column matmul, same psum style)
  attn = out2 * (1/den broadcast via rank-1 matmul)
  y = out_w @ attn + (out_w @ bv + ob) + x     (v-bias folded via sum(A)=1)
"""
import numpy as np

_CACHE: dict = {}

B, C, N = 4, 256, 4096
NQ = 2048          # queries per core
G = 8              # groups
GS = 32            # channels per group
EPS = 1e-5
SCALE = C ** -0.5


def _install_tile_drain_patch():
    """This container's walrus only supports ONE sync-wait command per
    instruction; TileContext's final drain carries one wait per live proc and
    fails codegen. Re-emit the waits as standalone sem-wait instructions."""
    import concourse.tile as tile_mod
    from concourse.vector_clock import ScopedClock

    if getattr(tile_mod.TileContext, "_ant_drain_patched", False):
        return

    def _patched(self, tick_clock, wait_clock):
        nc = self.nc
        probe = nc.sync.nop(nofuse=True)
        wait_clock.add_sem_waits(
            probe.ins, ScopedClock({None: tick_clock.global_clock})
        )
        waits = list(probe.ins.sync_info.on_wait or [])
        if len(waits) > 1:
            probe.ins.sync_info.on_wait = []
            num2h = {}
            assert self.sems is not None
            for h in self.sems.allocated().values():
                num2h[h.num] = h
            for h in self.sems.swdge_block():
                num2h.setdefault(h.num, h)
            for w in waits:
                h = num2h.get(w.id)
                assert h is not None, f"sem id {w.id} ({w.ant_name}) has no handle"
                nc.sync.wait_op(h, w.wait_value, "sem-ge")
        nc.sync.drain()
        nc.all_engine_barrier()
        assert self.sems is not None
        popped = nc._tile_sem_poison_stack.pop()
        assert popped is self._sem_poison
        nc.clear_and_free_semaphores(list(self.sems.allocated().values()))
        nc.all_engine_barrier()

    tile_mod.TileContext._drain_and_barrier = _patched
    tile_mod.TileContext._ant_drain_patched = True


def _split_multiwait_instructions(nc):
    """This walrus build supports only one sync-wait command per instruction.
    Hoist extra waits into standalone InstEventSemaphore waits on the same
    engine immediately before the instruction (sequential single waits are
    equivalent to one multi-wait)."""
    import concourse.mybir as mybir

    n = 0
    for f in nc.m.functions:
        for bb in f.blocks:
            new = []
            for ins in bb.instructions:
                si = ins.sync_info
                waits = list(si.on_wait) if si else []
                if len(waits) > 1:
                    for w in waits[:-1]:
                        nop = mybir.InstEventSemaphore(
                            name=f"WSPLIT-{n}", ins=[], outs=[])
                        n += 1
                        nop.engine = ins.engine
                        nop.sync_info = mybir.SyncInfo(on_wait=[w], on_update=[])
                        new.append(nop)
                    si.on_wait = [waits[-1]]
                new.append(ins)
            bb.instructions[:] = new


def _build():
    import concourse.bass as bass
    import concourse.tile as tile
    import concourse.mybir as mybir
    from contextlib import ExitStack

    _install_tile_drain_patch()

    f32 = mybir.dt.float32
    Act = mybir.ActivationFunctionType
    Alu = mybir.AluOpType

    nc = bass.Bass()
    x_kv = nc.dram_tensor("x_kv", [C, N], f32, kind="ExternalInput")
    qkv_wT = nc.dram_tensor("qkv_wT", [C, 3 * C], f32, kind="ExternalInput")
    qkv_b = nc.dram_tensor("qkv_b", [3 * C, 1], f32, kind="ExternalInput")
    norm_w = nc.dram_tensor("norm_w", [C, 1], f32, kind="ExternalInput")
    norm_b = nc.dram_tensor("norm_b", [C, 1], f32, kind="ExternalInput")
    out_wT = nc.dram_tensor("out_wT", [C, C], f32, kind="ExternalInput")
    out_b = nc.dram_tensor("out_b", [C, 1], f32, kind="ExternalInput")
    y = nc.dram_tensor("y", [C, NQ], f32, kind="ExternalOutput")

    with ExitStack() as ctx:
        tc = ctx.enter_context(tile.TileContext(nc))
        const = ctx.enter_context(tc.tile_pool(name="const", bufs=1))
        xhp = ctx.enter_context(tc.tile_pool(name="xhp", bufs=2))
        kp = ctx.enter_context(tc.tile_pool(name="kp", bufs=2))
        qp = ctx.enter_context(tc.tile_pool(name="qp", bufs=2))
        vtp = ctx.enter_context(tc.tile_pool(name="vtp", bufs=32))
        esp = ctx.enter_context(tc.tile_pool(name="esp", bufs=12))
        smal = ctx.enter_context(tc.tile_pool(name="smal", bufs=2))
        sb2 = ctx.enter_context(tc.tile_pool(name="sb2", bufs=2))
        psp = ctx.enter_context(tc.tile_pool(name="psp", bufs=3, space="PSUM"))
        pvp = ctx.enter_context(tc.tile_pool(name="pvp", bufs=2, space="PSUM"))
        denp = ctx.enter_context(tc.tile_pool(name="denp", bufs=1, space="PSUM"))
        finp = ctx.enter_context(tc.tile_pool(name="finp", bufs=2, space="PSUM"))

        # ---- constants ----
        ones_col = const.tile([128, 1], f32, name="ones_col")
        nc.vector.memset(ones_col, 1.0)
        ones_row = const.tile([1, 128], f32, name="ones_row")
        nc.vector.memset(ones_row, 1.0)
        ind_fwd = const.tile([128, 4], f32, name="ind_fwd")   # [ch, grp] one-hot
        nc.vector.memset(ind_fwd, 0.0)
        for g in range(4):
            nc.vector.memset(ind_fwd[g * GS:(g + 1) * GS, g:g + 1], 1.0)
        ind_bwd = const.tile([4, 128], f32, name="ind_bwd")   # [grp, ch] one-hot
        nc.vector.memset(ind_bwd[0:4, :], 0.0)
        for g in range(4):
            # engines can't start at partition g∉{0,32,64,96}; DMA can
            nc.sync.dma_start(out=ind_bwd[g:g + 1, g * GS:(g + 1) * GS],
                              in_=ones_row[0:1, 0:GS])
        eps_t = const.tile([128, 1], f32, name="eps_t")
        nc.vector.memset(eps_t, EPS)

        # ---- weights / biases ----
        wt, owt, nw, nb, ob = [], [], [], [], []
        for ci in range(2):
            w = const.tile([128, 3 * C], f32, name=f"wt{ci}")
            nc.sync.dma_start(out=w, in_=qkv_wT[ci * 128:(ci + 1) * 128, :])
            wt.append(w)
            o = const.tile([128, C], f32, name=f"owt{ci}")
            nc.sync.dma_start(out=o, in_=out_wT[ci * 128:(ci + 1) * 128, :])
            owt.append(o)
            t = const.tile([128, 1], f32, name=f"nw{ci}")
            nc.sync.dma_start(out=t, in_=norm_w[ci * 128:(ci + 1) * 128, :])
            nw.append(t)
            t = const.tile([128, 1], f32, name=f"nb{ci}")
            nc.sync.dma_start(out=t, in_=norm_b[ci * 128:(ci + 1) * 128, :])
            nb.append(t)
            t = const.tile([128, 1], f32, name=f"ob{ci}")
            nc.sync.dma_start(out=t, in_=out_b[ci * 128:(ci + 1) * 128, :])
            ob.append(t)
        qb = []
        for j in range(6):
            t = const.tile([128, 1], f32, name=f"qb{j}")
            nc.sync.dma_start(out=t, in_=qkv_b[j * 128:(j + 1) * 128, :])
            qb.append(t)

        # ---- load x ----
        xh = []
        for ct in range(2):
            xt = xhp.tile([128, N], f32, tag="xh", name=f"xh{ct}")
            for j in range(4):
                nc.sync.dma_start(
                    out=xt[:, j * 1024:(j + 1) * 1024],
                    in_=x_kv[ct * 128:(ct + 1) * 128, j * 1024:(j + 1) * 1024],
                )
            xh.append(xt)

        # ---- groupnorm stats -> per-channel affine (A, B) ----
        AB = []
        for ct in range(2):
            xr = xh[ct].rearrange("p (s f) -> p s f", f=512)
            stats = smal.tile([128, 8, 6], f32, tag="stats", name=f"stats{ct}")
            for s in range(8):
                nc.vector.bn_stats(out=stats[:, s, :], in_=xr[:, s, :])
            mv = smal.tile([128, 2], f32, tag="mv", name=f"mv{ct}")
            nc.vector.bn_aggr(out=mv, in_=stats)
            st2 = smal.tile([128, 2], f32, tag="st2", name=f"st2{ct}")
            nc.vector.tensor_copy(out=st2[:, 0:1], in_=mv[:, 0:1])
            nc.vector.tensor_mul(out=st2[:, 1:2], in0=mv[:, 0:1], in1=mv[:, 0:1])
            nc.vector.tensor_add(out=st2[:, 1:2], in0=st2[:, 1:2], in1=mv[:, 1:2])
            gp = psp.tile([4, 2], f32, tag="ps", name=f"gp{ct}")
            nc.tensor.matmul(gp, ind_fwd, st2, start=True, stop=True)
            gsb = smal.tile([4, 2], f32, tag="gsb", name=f"gsb{ct}")
            nc.scalar.mul(out=gsb, in_=gp, mul=1.0 / GS)  # group mean = avg of 32 channel means
            chp = psp.tile([128, 2], f32, tag="ps", name=f"chp{ct}")
            nc.tensor.matmul(chp, ind_bwd, gsb, start=True, stop=True)
            ch = smal.tile([128, 2], f32, tag="ch", name=f"ch{ct}")
            nc.vector.tensor_copy(out=ch, in_=chp)
            var = smal.tile([128, 1], f32, tag="var", name=f"var{ct}")
            nc.vector.tensor_mul(out=var, in0=ch[:, 0:1], in1=ch[:, 0:1])
            nc.vector.tensor_sub(out=var, in0=ch[:, 1:2], in1=var)
            std = smal.tile([128, 1], f32, tag="std", name=f"std{ct}")
            nc.scalar.activation(out=std, in_=var, func=Act.Sqrt, bias=eps_t)
            rstd = smal.tile([128, 1], f32, tag="rstd", name=f"rstd{ct}")
            nc.vector.reciprocal(out=rstd, in_=std)
            A = smal.tile([128, 1], f32, tag="A", name=f"A{ct}")
            nc.vector.tensor_mul(out=A, in0=nw[ct], in1=rstd)
            Bb = smal.tile([128, 1], f32, tag="B", name=f"B{ct}")
            nc.vector.tensor_mul(out=Bb, in0=ch[:, 0:1], in1=A)
            nc.vector.tensor_sub(out=Bb, in0=nb[ct], in1=Bb)
            AB.append((A, Bb))
        for ct in range(2):
            nc.vector.tensor_scalar(
                out=xh[ct], in0=xh[ct],
                scalar1=AB[ct][0], scalar2=AB[ct][1],
                op0=Alu.mult, op1=Alu.add,
            )

        # ---- q conv (queries = first NQ cols) ----
        q_sb = [qp.tile([128, NQ], f32, tag="q", name=f"q{co}") for co in range(2)]
        for co in range(2):
            for nch in range(NQ // 512):
                pq = psp.tile([128, 512], f32, tag="ps", name=f"pq{co}_{nch}")
                nc.tensor.matmul(pq, wt[0][:, co * 128:(co + 1) * 128],
                                 xh[0][:, nch * 512:(nch + 1) * 512],
                                 start=True, stop=False)
                nc.tensor.matmul(pq, wt[1][:, co * 128:(co + 1) * 128],
                                 xh[1][:, nch * 512:(nch + 1) * 512],
                                 start=False, stop=True)
                nc.vector.tensor_scalar_add(
                    out=q_sb[co][:, nch * 512:(nch + 1) * 512],
                    in0=pq, scalar1=qb[co])

        # ---- k conv ----
        k_sb = [kp.tile([128, N], f32, tag="k", name=f"k{co}") for co in range(2)]
        for co in range(2):
            for nch in range(N // 512):
                pk = psp.tile([128, 512], f32, tag="ps", name=f"pk{co}_{nch}")
                nc.tensor.matmul(pk, wt[0][:, 256 + co * 128:256 + (co + 1) * 128],
                                 xh[0][:, nch * 512:(nch + 1) * 512],
                                 start=True, stop=False)
                nc.tensor.matmul(pk, wt[1][:, 256 + co * 128:256 + (co + 1) * 128],
                                 xh[1][:, nch * 512:(nch + 1) * 512],
                                 start=False, stop=True)
                nc.vector.tensor_scalar_add(
                    out=k_sb[co][:, nch * 512:(nch + 1) * 512],
                    in0=pk, scalar1=qb[2 + co])

        # ---- vT conv (no bias; folded into effb) ----
        vt_sb = []
        for mt in range(32):
            pvt = psp.tile([128, 256], f32, tag="ps", name=f"pvt{mt}")
            nc.tensor.matmul(pvt, xh[0][:, mt * 128:(mt + 1) * 128],
                             wt[0][:, 512:768], start=True, stop=False)
            nc.tensor.matmul(pvt, xh[1][:, mt * 128:(mt + 1) * 128],
                             wt[1][:, 512:768], start=False, stop=True)
            vt = vtp.tile([128, 256], f32, tag="vt", name=f"vt{mt}")
            nc.vector.tensor_copy(out=vt, in_=pvt)
            vt_sb.append(vt)

        # ---- effective output bias: out_w @ v_bias + out_b ----
        effb = []
        for co in range(2):
            pe_ = psp.tile([128, 1], f32, tag="ps", name=f"peffb{co}")
            nc.tensor.matmul(pe_, owt[0][:, co * 128:(co + 1) * 128], qb[4],
                             start=True, stop=False)
            nc.tensor.matmul(pe_, owt[1][:, co * 128:(co + 1) * 128], qb[5],
                             start=False, stop=True)
            e = smal.tile([128, 1], f32, tag="effb", name=f"effb{co}")
            nc.vector.tensor_add(out=e, in0=pe_, in1=ob[co])
            effb.append(e)

        # ---- attention (software-pipelined over key chunks) ----
        def st_chunk(t, mch):
            ps_s = psp.tile([128, 512], f32, tag="ps", name=f"s_{t}_{mch}")
            nc.tensor.matmul(ps_s, k_sb[0][:, mch * 128:(mch + 1) * 128],
                             q_sb[0][:, t * 512:(t + 1) * 512],
                             start=True, stop=False)
            nc.tensor.matmul(ps_s, k_sb[1][:, mch * 128:(mch + 1) * 128],
                             q_sb[1][:, t * 512:(t + 1) * 512],
                             start=False, stop=True)
            es = esp.tile([128, 512], f32, tag="es", name=f"es_{t}_{mch}")
            nc.scalar.activation(out=es, in_=ps_s, func=Act.Exp, scale=SCALE)
            return es

        NT = NQ // 512
        for t in range(NT):
            pv = [pvp.tile([128, 512], f32, tag="pv", name=f"pv_{t}_{co}")
                  for co in range(2)]
            den = denp.tile([1, 512], f32, tag="den", name=f"den_{t}")
            es_q = {0: st_chunk(t, 0), 1: st_chunk(t, 1)}
            for mch in range(32):
                es = es_q.pop(mch)
                st, sp = (mch == 0), (mch == 31)
                nc.tensor.matmul(pv[0], vt_sb[mch][:, 0:128], es, start=st, stop=sp)
                nc.tensor.matmul(pv[1], vt_sb[mch][:, 128:256], es, start=st, stop=sp)
                nc.tensor.matmul(den, ones_col, es, start=st, stop=sp)
                if mch + 2 < 32:
                    es_q[mch + 2] = st_chunk(t, mch + 2)
            # epilogue
            dsb = smal.tile([1, 512], f32, tag="dsb", name=f"dsb_{t}")
            nc.vector.tensor_copy(out=dsb, in_=den)
            rdsb = smal.tile([1, 512], f32, tag="rdsb", name=f"rdsb_{t}")
            nc.vector.reciprocal(out=rdsb, in_=dsb)
            bc = psp.tile([128, 512], f32, tag="ps", name=f"bc_{t}")
            nc.tensor.matmul(bc, ones_row, rdsb, start=True, stop=True)
            rdb = sb2.tile([128, 512], f32, tag="rdb", name=f"rdb_{t}")
            nc.scalar.copy(out=rdb, in_=bc)
            attn = []
            for co in range(2):
                a = sb2.tile([128, 512], f32, tag=f"attn{co}", name=f"attn_{t}_{co}")
                nc.vector.tensor_mul(out=a, in0=pv[co], in1=rdb)
                attn.append(a)
            for co in range(2):
                fin = finp.tile([128, 512], f32, tag="fin", name=f"fin_{t}_{co}")
                nc.tensor.matmul(fin, owt[0][:, co * 128:(co + 1) * 128], attn[0],
                                 start=True, stop=False)
                nc.tensor.matmul(fin, owt[1][:, co * 128:(co + 1) * 128], attn[1],
                                 start=False, stop=True)
                fsb = sb2.tile([128, 512], f32, tag=f"fsb{co}", name=f"f_{t}_{co}")
                nc.vector.tensor_scalar_add(out=fsb, in0=fin, scalar1=effb[co])
                rsb = sb2.tile([128, 512], f32, tag=f"rsb{co}", name=f"r_{t}_{co}")
                nc.sync.dma_start(
                    out=rsb,
                    in_=x_kv[co * 128:(co + 1) * 128, t * 512:(t + 1) * 512])
                ysb = sb2.tile([128, 512], f32, tag=f"ysb{co}", name=f"y_{t}_{co}")
                nc.vector.tensor_add(out=ysb, in0=fsb, in1=rsb)
                nc.sync.dma_start(
                    out=y[co * 128:(co + 1) * 128, t * 512:(t + 1) * 512],
                    in_=ysb)

    _split_multiwait_instructions(nc)
    return nc


def _get_nc():
    if "nc" not in _CACHE:
        _CACHE["nc"] = _build()
    return _CACHE["nc"]


def kernel(x, norm_w, norm_b, qkv_w, qkv_b, out_w, out_b):
    from concourse.bass_utils import run_bass_kernel_spmd

    nc = _get_nc()
    xr = np.ascontiguousarray(np.asarray(x, dtype=np.float32).reshape(B, C, N))
    wT = np.ascontiguousarray(np.asarray(qkv_w, np.float32).T)
    owT = np.ascontiguousarray(np.asarray(out_w, np.float32).T)
    qb2 = np.ascontiguousarray(np.asarray(qkv_b, np.float32).reshape(3 * C, 1))
    nw2 = np.ascontiguousarray(np.asarray(norm_w, np.float32).reshape(C, 1))
    nb2 = np.ascontiguousarray(np.asarray(norm_b, np.float32).reshape(C, 1))
    ob2 = np.ascontiguousarray(np.asarray(out_b, np.float32).reshape(C, 1))

    in_maps = []
    for c in range(8):
        b, h = divmod(c, 2)
        off = h * NQ
        xb = xr[b]
        if off:
            xroll = np.ascontiguousarray(
                np.concatenate([xb[:, off:], xb[:, :off]], axis=1))
        else:
            xroll = xb
        in_maps.append({
            "x_kv": xroll, "qkv_wT": wT, "qkv_b": qb2,
            "norm_w": nw2, "norm_b": nb2, "out_wT": owT, "out_b": ob2,
        })

    res = run_bass_kernel_spmd(nc, in_maps, core_ids=list(range(8)))
    out = np.empty((B, C, N), np.float32)
    for c in range(8):
        b, h = divmod(c, 2)
        off = h * NQ
        out[b][:, off:off + NQ] = res.results[c]["y"]
    return out.reshape(B, C, 16, 16, 16)


# revision 10
# speedup vs baseline: 2.6955x; 2.6955x over previous
"""AttentionBlock3D (GroupNorm + single-head attention over 4096 tokens + residual)
on 8 Trainium2 NeuronCores.

Sharding: core c handles sample b=c//2 and query half h=c%2 (2048 queries).
The host rotates each sample's token axis so that each core's queries are
columns 0..2048 of its x_kv input (attention is permutation-invariant over
keys, and groupnorm stats are permutation-invariant), letting a single SPMD
program serve all 8 cores with no dynamic offsets.

Device-side math per core (all fp32):
  h = groupnorm(x)                       (bn_stats + indicator-matmul group reduce)
  q = Wq h + bq  [256,2048]              (host passes qkv_w^T; no on-chip transposes)
  k = Wk h + bk  [256,4096]
  vT = (Wv h)^T  [4096,256]              (computed directly in transposed layout)
  S^T tiles  [128 keys, 512 queries] = k_chunk^T-free matmuls
  es = exp(S^T * C^-0.5)                 (no max subtraction: |scores*scale| <~ 8)
  out2[c,q] = sum_m vT[m,c] es[m,q]      (v chunks stationary, es streams)
  den[q]    = sum_m es[m,q]              (ones-column matmul, same psum style)
  attn = out2 * (1/den broadcast via rank-1 matmul)
  y = out_w @ attn + (out_w @ bv + ob) + x     (v-bias folded via sum(A)=1)
"""
import numpy as np

_CACHE: dict = {}

B, C, N = 4, 256, 4096
NQ = 2048          # queries per core
G = 8              # groups
GS = 32            # channels per group
EPS = 1e-5
SCALE = C ** -0.5


def _install_tile_drain_patch():
    """This container's walrus only supports ONE sync-wait command per
    instruction; TileContext's final drain carries one wait per live proc and
    fails codegen. Re-emit the waits as standalone sem-wait instructions."""
    import concourse.tile as tile_mod
    from concourse.vector_clock import ScopedClock

    if getattr(tile_mod.TileContext, "_ant_drain_patched", False):
        return

    def _patched(self, tick_clock, wait_clock):
        nc = self.nc
        probe = nc.sync.nop(nofuse=True)
        wait_clock.add_sem_waits(
            probe.ins, ScopedClock({None: tick_clock.global_clock})
        )
        waits = list(probe.ins.sync_info.on_wait or [])
        if len(waits) > 1:
            probe.ins.sync_info.on_wait = []
            num2h = {}
            assert self.sems is not None
            for h in self.sems.allocated().values():
                num2h[h.num] = h
            for h in self.sems.swdge_block():
                num2h.setdefault(h.num, h)
            for w in waits:
                h = num2h.get(w.id)
                assert h is not None, f"sem id {w.id} ({w.ant_name}) has no handle"
                nc.sync.wait_op(h, w.wait_value, "sem-ge")
        nc.sync.drain()
        nc.all_engine_barrier()
        assert self.sems is not None
        popped = nc._tile_sem_poison_stack.pop()
        assert popped is self._sem_poison
        nc.clear_and_free_semaphores(list(self.sems.allocated().values()))
        nc.all_engine_barrier()

    tile_mod.TileContext._drain_and_barrier = _patched
    tile_mod.TileContext._ant_drain_patched = True


def _split_multiwait_instructions(nc):
    """This walrus build supports only one sync-wait command per instruction.
    Hoist extra waits into standalone InstEventSemaphore waits on the same
    engine immediately before the instruction (sequential single waits are
    equivalent to one multi-wait)."""
    import concourse.mybir as mybir

    n = 0
    for f in nc.m.functions:
        for bb in f.blocks:
            new = []
            for ins in bb.instructions:
                si = ins.sync_info
                waits = list(si.on_wait) if si else []
                if len(waits) > 1:
                    for w in waits[:-1]:
                        nop = mybir.InstEventSemaphore(
                            name=f"WSPLIT-{n}", ins=[], outs=[])
                        n += 1
                        nop.engine = ins.engine
                        nop.sync_info = mybir.SyncInfo(on_wait=[w], on_update=[])
                        new.append(nop)
                    si.on_wait = [waits[-1]]
                new.append(ins)
            bb.instructions[:] = new


def _build():
    import concourse.bass as bass
    import concourse.tile as tile
    import concourse.mybir as mybir
    from contextlib import ExitStack

    _install_tile_drain_patch()

    f32 = mybir.dt.float32
    f16 = mybir.dt.float16
    Act = mybir.ActivationFunctionType
    Alu = mybir.AluOpType

    nc = bass.Bass()
    x_kv = nc.dram_tensor("x_kv", [C, N], f32, kind="ExternalInput")
    qkv_wT = nc.dram_tensor("qkv_wT", [C, 3 * C], f32, kind="ExternalInput")
    qkv_b = nc.dram_tensor("qkv_b", [3 * C, 1], f32, kind="ExternalInput")
    norm_w = nc.dram_tensor("norm_w", [C, 1], f32, kind="ExternalInput")
    norm_b = nc.dram_tensor("norm_b", [C, 1], f32, kind="ExternalInput")
    out_wT = nc.dram_tensor("out_wT", [C, C], f32, kind="ExternalInput")
    out_b = nc.dram_tensor("out_b", [C, 1], f32, kind="ExternalInput")
    y = nc.dram_tensor("y", [C, NQ], f32, kind="ExternalOutput")

    with ExitStack() as ctx:
        tc = ctx.enter_context(tile.TileContext(nc))
        const = ctx.enter_context(tc.tile_pool(name="const", bufs=1))
        xhp = ctx.enter_context(tc.tile_pool(name="xhp", bufs=2))
        kp = ctx.enter_context(tc.tile_pool(name="kp", bufs=2))
        qp = ctx.enter_context(tc.tile_pool(name="qp", bufs=2))
        vtp = ctx.enter_context(tc.tile_pool(name="vtp", bufs=32))
        esp = ctx.enter_context(tc.tile_pool(name="esp", bufs=16))
        smal = ctx.enter_context(tc.tile_pool(name="smal", bufs=2))
        sb2 = ctx.enter_context(tc.tile_pool(name="sb2", bufs=2))
        psp = ctx.enter_context(tc.tile_pool(name="psp", bufs=3, space="PSUM"))
        pvp = ctx.enter_context(tc.tile_pool(name="pvp", bufs=2, space="PSUM"))
        denp = ctx.enter_context(tc.tile_pool(name="denp", bufs=1, space="PSUM"))
        finp = ctx.enter_context(tc.tile_pool(name="finp", bufs=2, space="PSUM"))

        # ---- constants ----
        ones_col = const.tile([128, 1], f16, name="ones_col")
        nc.vector.memset(ones_col, 1.0)
        ones_row = const.tile([1, 128], f32, name="ones_row")
        nc.vector.memset(ones_row, 1.0)
        ind_fwd = const.tile([128, 4], f32, name="ind_fwd")   # [ch, grp] one-hot
        nc.vector.memset(ind_fwd, 0.0)
        for g in range(4):
            nc.vector.memset(ind_fwd[g * GS:(g + 1) * GS, g:g + 1], 1.0)
        ind_bwd = const.tile([4, 128], f32, name="ind_bwd")   # [grp, ch] one-hot
        nc.vector.memset(ind_bwd[0:4, :], 0.0)
        for g in range(4):
            # engines can't start at partition g∉{0,32,64,96}; DMA can
            nc.sync.dma_start(out=ind_bwd[g:g + 1, g * GS:(g + 1) * GS],
                              in_=ones_row[0:1, 0:GS])
        eps_t = const.tile([128, 1], f32, name="eps_t")
        nc.vector.memset(eps_t, EPS)

        # ---- weights / biases ----
        wt, owt, nw, nb, ob = [], [], [], [], []
        for ci in range(2):
            w = const.tile([128, 3 * C], f32, name=f"wt{ci}")
            nc.sync.dma_start(out=w, in_=qkv_wT[ci * 128:(ci + 1) * 128, :])
            wt.append(w)
            o = const.tile([128, C], f32, name=f"owt{ci}")
            nc.sync.dma_start(out=o, in_=out_wT[ci * 128:(ci + 1) * 128, :])
            owt.append(o)
            t = const.tile([128, 1], f32, name=f"nw{ci}")
            nc.sync.dma_start(out=t, in_=norm_w[ci * 128:(ci + 1) * 128, :])
            nw.append(t)
            t = const.tile([128, 1], f32, name=f"nb{ci}")
            nc.sync.dma_start(out=t, in_=norm_b[ci * 128:(ci + 1) * 128, :])
            nb.append(t)
            t = const.tile([128, 1], f32, name=f"ob{ci}")
            nc.sync.dma_start(out=t, in_=out_b[ci * 128:(ci + 1) * 128, :])
            ob.append(t)
        qb = []
        for j in range(6):
            t = const.tile([128, 1], f32, name=f"qb{j}")
            nc.sync.dma_start(out=t, in_=qkv_b[j * 128:(j + 1) * 128, :])
            qb.append(t)

        # ---- fp16 casts of matmul weights (PE fp32 runs at half rate,
        #      lowered to 2 HW passes; fp16 streams at full rate) ----
        wt16, owt16 = [], []
        for ci in range(2):
            w16 = const.tile([128, 3 * C], f16, name=f"wt16_{ci}")
            nc.vector.tensor_copy(out=w16, in_=wt[ci])
            wt16.append(w16)
            o16 = const.tile([128, C], f16, name=f"owt16_{ci}")
            nc.vector.tensor_copy(out=o16, in_=owt[ci])
            owt16.append(o16)
        qb16v = []
        for j in (4, 5):
            t16 = const.tile([128, 1], f16, name=f"qb16_{j}")
            nc.vector.tensor_copy(out=t16, in_=qb[j])
            qb16v.append(t16)

        # ---- load x ----
        xh = []
        for ct in range(2):
            xt = xhp.tile([128, N], f32, tag="xh", name=f"xh{ct}")
            for j in range(4):
                nc.sync.dma_start(
                    out=xt[:, j * 1024:(j + 1) * 1024],
                    in_=x_kv[ct * 128:(ct + 1) * 128, j * 1024:(j + 1) * 1024],
                )
            xh.append(xt)

        # ---- groupnorm stats -> per-channel affine (A, B) ----
        AB = []
        for ct in range(2):
            xr = xh[ct].rearrange("p (s f) -> p s f", f=512)
            stats = smal.tile([128, 8, 6], f32, tag="stats", name=f"stats{ct}")
            for s in range(8):
                nc.vector.bn_stats(out=stats[:, s, :], in_=xr[:, s, :])
            mv = smal.tile([128, 2], f32, tag="mv", name=f"mv{ct}")
            nc.vector.bn_aggr(out=mv, in_=stats)
            st2 = smal.tile([128, 2], f32, tag="st2", name=f"st2{ct}")
            nc.vector.tensor_copy(out=st2[:, 0:1], in_=mv[:, 0:1])
            nc.vector.tensor_mul(out=st2[:, 1:2], in0=mv[:, 0:1], in1=mv[:, 0:1])
            nc.vector.tensor_add(out=st2[:, 1:2], in0=st2[:, 1:2], in1=mv[:, 1:2])
            gp = psp.tile([4, 2], f32, tag="ps", name=f"gp{ct}")
            nc.tensor.matmul(gp, ind_fwd, st2, start=True, stop=True)
            gsb = smal.tile([4, 2], f32, tag="gsb", name=f"gsb{ct}")
            nc.scalar.mul(out=gsb, in_=gp, mul=1.0 / GS)  # group mean = avg of 32 channel means
            chp = psp.tile([128, 2], f32, tag="ps", name=f"chp{ct}")
            nc.tensor.matmul(chp, ind_bwd, gsb, start=True, stop=True)
            ch = smal.tile([128, 2], f32, tag="ch", name=f"ch{ct}")
            nc.vector.tensor_copy(out=ch, in_=chp)
            var = smal.tile([128, 1], f32, tag="var", name=f"var{ct}")
            nc.vector.tensor_mul(out=var, in0=ch[:, 0:1], in1=ch[:, 0:1])
            nc.vector.tensor_sub(out=var, in0=ch[:, 1:2], in1=var)
            std = smal.tile([128, 1], f32, tag="std", name=f"std{ct}")
            nc.scalar.activation(out=std, in_=var, func=Act.Sqrt, bias=eps_t)
            rstd = smal.tile([128, 1], f32, tag="rstd", name=f"rstd{ct}")
            nc.vector.reciprocal(out=rstd, in_=std)
            A = smal.tile([128, 1], f32, tag="A", name=f"A{ct}")
            nc.vector.tensor_mul(out=A, in0=nw[ct], in1=rstd)
            Bb = smal.tile([128, 1], f32, tag="B", name=f"B{ct}")
            nc.vector.tensor_mul(out=Bb, in0=ch[:, 0:1], in1=A)
            nc.vector.tensor_sub(out=Bb, in0=nb[ct], in1=Bb)
            AB.append((A, Bb))
        h16 = []
        for ct in range(2):
            ht = xhp.tile([128, N], f16, tag="h16", name=f"h16_{ct}")
            nc.vector.tensor_scalar(
                out=ht, in0=xh[ct],
                scalar1=AB[ct][0], scalar2=AB[ct][1],
                op0=Alu.mult, op1=Alu.add,
            )
            h16.append(ht)

        # ---- q conv (queries = first NQ cols) ----
        q_sb = [qp.tile([128, NQ], f16, tag="q", name=f"q{co}") for co in range(2)]
        for co in range(2):
            for nch in range(NQ // 512):
                pq = psp.tile([128, 512], f32, tag="ps", name=f"pq{co}_{nch}")
                nc.tensor.matmul(pq, wt16[0][:, co * 128:(co + 1) * 128],
                                 h16[0][:, nch * 512:(nch + 1) * 512],
                                 start=True, stop=False)
                nc.tensor.matmul(pq, wt16[1][:, co * 128:(co + 1) * 128],
                                 h16[1][:, nch * 512:(nch + 1) * 512],
                                 start=False, stop=True)
                nc.vector.tensor_scalar_add(
                    out=q_sb[co][:, nch * 512:(nch + 1) * 512],
                    in0=pq, scalar1=qb[co])

        # ---- k conv ----
        k_sb = [kp.tile([128, N], f16, tag="k", name=f"k{co}") for co in range(2)]
        for co in range(2):
            for nch in range(N // 512):
                pk = psp.tile([128, 512], f32, tag="ps", name=f"pk{co}_{nch}")
                nc.tensor.matmul(pk, wt16[0][:, 256 + co * 128:256 + (co + 1) * 128],
                                 h16[0][:, nch * 512:(nch + 1) * 512],
                                 start=True, stop=False)
                nc.tensor.matmul(pk, wt16[1][:, 256 + co * 128:256 + (co + 1) * 128],
                                 h16[1][:, nch * 512:(nch + 1) * 512],
                                 start=False, stop=True)
                nc.vector.tensor_scalar_add(
                    out=k_sb[co][:, nch * 512:(nch + 1) * 512],
                    in0=pk, scalar1=qb[2 + co])

        # ---- vT conv (no bias; folded into effb) ----
        vt_sb = []
        for mt in range(32):
            pvt = psp.tile([128, 256], f32, tag="ps", name=f"pvt{mt}")
            nc.tensor.matmul(pvt, h16[0][:, mt * 128:(mt + 1) * 128],
                             wt16[0][:, 512:768], start=True, stop=False)
            nc.tensor.matmul(pvt, h16[1][:, mt * 128:(mt + 1) * 128],
                             wt16[1][:, 512:768], start=False, stop=True)
            vt = vtp.tile([128, 256], f16, tag="vt", name=f"vt{mt}")
            nc.vector.tensor_copy(out=vt, in_=pvt)
            vt_sb.append(vt)

        # ---- effective output bias: out_w @ v_bias + out_b ----
        effb = []
        for co in range(2):
            pe_ = psp.tile([128, 1], f32, tag="ps", name=f"peffb{co}")
            nc.tensor.matmul(pe_, owt16[0][:, co * 128:(co + 1) * 128], qb16v[0],
                             start=True, stop=False)
            nc.tensor.matmul(pe_, owt16[1][:, co * 128:(co + 1) * 128], qb16v[1],
                             start=False, stop=True)
            e = smal.tile([128, 1], f32, tag="effb", name=f"effb{co}")
            nc.vector.tensor_add(out=e, in0=pe_, in1=ob[co])
            effb.append(e)

        # ---- attention (software-pipelined over key chunks) ----
        def st_chunk(t, mch):
            ps_s = psp.tile([128, 512], f32, tag="ps", name=f"s_{t}_{mch}")
            nc.tensor.matmul(ps_s, k_sb[0][:, mch * 128:(mch + 1) * 128],
                             q_sb[0][:, t * 512:(t + 1) * 512],
                             start=True, stop=False)
            nc.tensor.matmul(ps_s, k_sb[1][:, mch * 128:(mch + 1) * 128],
                             q_sb[1][:, t * 512:(t + 1) * 512],
                             start=False, stop=True)
            es = esp.tile([128, 512], f16, tag="es", name=f"es_{t}_{mch}")
            nc.scalar.activation(out=es, in_=ps_s, func=Act.Exp, scale=SCALE)
            return es

        NT = NQ // 512
        for t in range(NT):
            pv = [pvp.tile([128, 512], f32, tag="pv", name=f"pv_{t}_{co}")
                  for co in range(2)]
            den = denp.tile([1, 512], f32, tag="den", name=f"den_{t}")
            es_q = {0: st_chunk(t, 0), 1: st_chunk(t, 1)}
            for mch in range(32):
                es = es_q.pop(mch)
                st, sp = (mch == 0), (mch == 31)
                nc.tensor.matmul(pv[0], vt_sb[mch][:, 0:128], es, start=st, stop=sp)
                nc.tensor.matmul(pv[1], vt_sb[mch][:, 128:256], es, start=st, stop=sp)
                nc.tensor.matmul(den, ones_col, es, start=st, stop=sp)
                if mch + 2 < 32:
                    es_q[mch + 2] = st_chunk(t, mch + 2)
            # epilogue
            dsb = smal.tile([1, 512], f32, tag="dsb", name=f"dsb_{t}")
            nc.vector.tensor_copy(out=dsb, in_=den)
            rdsb = smal.tile([1, 512], f32, tag="rdsb", name=f"rdsb_{t}")
            nc.vector.reciprocal(out=rdsb, in_=dsb)
            bc = psp.tile([128, 512], f32, tag="ps", name=f"bc_{t}")
            nc.tensor.matmul(bc, ones_row, rdsb, start=True, stop=True)
            rdb = sb2.tile([128, 512], f32, tag="rdb", name=f"rdb_{t}")
            nc.scalar.copy(out=rdb, in_=bc)
            attn = []
            for co in range(2):
                a = sb2.tile([128, 512], f16, tag=f"attn{co}", name=f"attn_{t}_{co}")
                nc.vector.tensor_mul(out=a, in0=pv[co], in1=rdb)
                attn.append(a)
            for co in range(2):
                fin = finp.tile([128, 512], f32, tag="fin", name=f"fin_{t}_{co}")
                nc.tensor.matmul(fin, owt16[0][:, co * 128:(co + 1) * 128], attn[0],
                                 start=True, stop=False)
                nc.tensor.matmul(fin, owt16[1][:, co * 128:(co + 1) * 128], attn[1],
                                 start=False, stop=True)
                fsb = sb2.tile([128, 512], f32, tag=f"fsb{co}", name=f"f_{t}_{co}")
                nc.vector.tensor_scalar_add(out=fsb, in0=fin, scalar1=effb[co])
                rsb = sb2.tile([128, 512], f32, tag=f"rsb{co}", name=f"r_{t}_{co}")
                nc.sync.dma_start(
                    out=rsb,
                    in_=x_kv[co * 128:(co + 1) * 128, t * 512:(t + 1) * 512])
                ysb = sb2.tile([128, 512], f32, tag=f"ysb{co}", name=f"y_{t}_{co}")
                nc.vector.tensor_add(out=ysb, in0=fsb, in1=rsb)
                nc.sync.dma_start(
                    out=y[co * 128:(co + 1) * 128, t * 512:(t + 1) * 512],
                    in_=ysb)

    _split_multiwait_instructions(nc)
    return nc


def _get_nc():
    if "nc" not in _CACHE:
        _CACHE["nc"] = _build()
    return _CACHE["nc"]


def kernel(x, norm_w, norm_b, qkv_w, qkv_b, out_w, out_b):
    from concourse.bass_utils import run_bass_kernel_spmd

    nc = _get_nc()
    xr = np.ascontiguousarray(np.asarray(x, dtype=np.float32).reshape(B, C, N))
    wT = np.ascontiguousarray(np.asarray(qkv_w, np.float32).T)
    owT = np.ascontiguousarray(np.asarray(out_w, np.float32).T)
    qb2 = np.ascontiguousarray(np.asarray(qkv_b, np.float32).reshape(3 * C, 1))
    nw2 = np.ascontiguousarray(np.asarray(norm_w, np.float32).reshape(C, 1))
    nb2 = np.ascontiguousarray(np.asarray(norm_b, np.float32).reshape(C, 1))
    ob2 = np.ascontiguousarray(np.asarray(out_b, np.float32).reshape(C, 1))

    in_maps = []
    for c in range(8):
        b, h = divmod(c, 2)
        off = h * NQ
        xb = xr[b]
        if off:
            xroll = np.ascontiguousarray(
                np.concatenate([xb[:, off:], xb[:, :off]], axis=1))
        else:
            xroll = xb
        in_maps.append({
            "x_kv": xroll, "qkv_wT": wT, "qkv_b": qb2,
            "norm_w": nw2, "norm_b": nb2, "out_wT": owT, "out_b": ob2,
        })

    res = run_bass_kernel_spmd(nc, in_maps, core_ids=list(range(8)))
    out = np.empty((B, C, N), np.float32)
    for c in range(8):
        b, h = divmod(c, 2)
        off = h * NQ
        out[b][:, off:off + NQ] = res.results[c]["y"]
    return out.reshape(B, C, 16, 16, 16)


# revision 11
# speedup vs baseline: 2.9659x; 1.1003x over previous
"""AttentionBlock3D (GroupNorm + single-head attention over 4096 tokens + residual)
on 8 Trainium2 NeuronCores.

Sharding: core c handles sample b=c//2 and query half h=c%2 (2048 queries).
The host rotates each sample's token axis so that each core's queries are
columns 0..2048 of its x_kv input (attention is permutation-invariant over
keys, and groupnorm stats are permutation-invariant), letting a single SPMD
program serve all 8 cores with no dynamic offsets.

Device-side math per core:
  h = groupnorm(x)                   fp32 stats (bn_stats + indicator-matmul
                                     cross-partition group reduce), fp16 output
  q = Wq h + bq  [256,2048] fp16     (host passes qkv_w^T; no on-chip transposes)
  k = Wk h + bk  [256,4096] fp16
  vT = (Wv h)^T  [4096,256] fp16     (computed directly in transposed layout)
  S^T tiles [128 keys, 512 queries] -> es = exp(S^T * C^-0.5) fp16
                                     (no max subtraction: |scores*scale| <~ 8)
  pv[c,q]  = sum_m vT[m,c] es[m,q]   (v chunks stationary, es streams, fp32 psum)
  den[q]   = sum_m es[m,q]           (ones-column matmul)
  fin      = out_w @ pv              (normalization deferred: a per-query column
                                      scale commutes through the projection)
  y = fin * (1/den bcast) + (out_w @ bv + ob) + x   (v-bias folded via sum(A)=1)

All matmuls run in fp16 (fp32 on the PE lowers to 2 HW passes at half rate);
PSUM accumulation is fp32 throughout, softmax statistics are fp32.
"""
import numpy as np

_CACHE: dict = {}

B, C, N = 4, 256, 4096
NQ = 2048          # queries per core
G = 8              # groups
GS = 32            # channels per group
EPS = 1e-5
SCALE = C ** -0.5


def _install_tile_drain_patch():
    """This container's walrus only supports ONE sync-wait command per
    instruction; TileContext's final drain carries one wait per live proc and
    fails codegen. Re-emit the waits as standalone sem-wait instructions."""
    import concourse.tile as tile_mod
    from concourse.vector_clock import ScopedClock

    if getattr(tile_mod.TileContext, "_ant_drain_patched", False):
        return

    def _patched(self, tick_clock, wait_clock):
        nc = self.nc
        probe = nc.sync.nop(nofuse=True)
        wait_clock.add_sem_waits(
            probe.ins, ScopedClock({None: tick_clock.global_clock})
        )
        waits = list(probe.ins.sync_info.on_wait or [])
        if len(waits) > 1:
            probe.ins.sync_info.on_wait = []
            num2h = {}
            assert self.sems is not None
            for h in self.sems.allocated().values():
                num2h[h.num] = h
            for h in self.sems.swdge_block():
                num2h.setdefault(h.num, h)
            for w in waits:
                h = num2h.get(w.id)
                assert h is not None, f"sem id {w.id} ({w.ant_name}) has no handle"
                nc.sync.wait_op(h, w.wait_value, "sem-ge")
        nc.sync.drain()
        nc.all_engine_barrier()
        assert self.sems is not None
        popped = nc._tile_sem_poison_stack.pop()
        assert popped is self._sem_poison
        nc.clear_and_free_semaphores(list(self.sems.allocated().values()))
        nc.all_engine_barrier()

    tile_mod.TileContext._drain_and_barrier = _patched
    tile_mod.TileContext._ant_drain_patched = True


def _split_multiwait_instructions(nc):
    """This walrus build supports only one sync-wait command per instruction.
    Hoist extra waits into standalone InstEventSemaphore waits on the same
    engine immediately before the instruction (sequential single waits are
    equivalent to one multi-wait)."""
    import concourse.mybir as mybir

    n = 0
    for f in nc.m.functions:
        for bb in f.blocks:
            new = []
            for ins in bb.instructions:
                si = ins.sync_info
                waits = list(si.on_wait) if si else []
                if len(waits) > 1:
                    for w in waits[:-1]:
                        nop = mybir.InstEventSemaphore(
                            name=f"WSPLIT-{n}", ins=[], outs=[])
                        n += 1
                        nop.engine = ins.engine
                        nop.sync_info = mybir.SyncInfo(on_wait=[w], on_update=[])
                        new.append(nop)
                    si.on_wait = [waits[-1]]
                new.append(ins)
            bb.instructions[:] = new


def _build():
    import concourse.bass as bass
    import concourse.tile as tile
    import concourse.mybir as mybir
    from contextlib import ExitStack

    _install_tile_drain_patch()

    f32 = mybir.dt.float32
    f16 = mybir.dt.float16
    Act = mybir.ActivationFunctionType
    Alu = mybir.AluOpType

    nc = bass.Bass()
    x_kv = nc.dram_tensor("x_kv", [C, N], f32, kind="ExternalInput")
    qkv_wT = nc.dram_tensor("qkv_wT", [C, 3 * C], f32, kind="ExternalInput")
    qkv_b = nc.dram_tensor("qkv_b", [3 * C, 1], f32, kind="ExternalInput")
    norm_w = nc.dram_tensor("norm_w", [C, 1], f32, kind="ExternalInput")
    norm_b = nc.dram_tensor("norm_b", [C, 1], f32, kind="ExternalInput")
    out_wT = nc.dram_tensor("out_wT", [C, C], f32, kind="ExternalInput")
    out_b = nc.dram_tensor("out_b", [C, 1], f32, kind="ExternalInput")
    y = nc.dram_tensor("y", [C, NQ], f32, kind="ExternalOutput")

    with ExitStack() as ctx:
        tc = ctx.enter_context(tile.TileContext(nc))
        const = ctx.enter_context(tc.tile_pool(name="const", bufs=1))
        xqp = ctx.enter_context(tc.tile_pool(name="xqp", bufs=8))
        hp = ctx.enter_context(tc.tile_pool(name="hp", bufs=8))
        kp = ctx.enter_context(tc.tile_pool(name="kp", bufs=2))
        qp = ctx.enter_context(tc.tile_pool(name="qp", bufs=2))
        vtp = ctx.enter_context(tc.tile_pool(name="vtp", bufs=32))
        esp = ctx.enter_context(tc.tile_pool(name="esp", bufs=16))
        smal = ctx.enter_context(tc.tile_pool(name="smal", bufs=2))
        sb2 = ctx.enter_context(tc.tile_pool(name="sb2", bufs=2))
        psp = ctx.enter_context(tc.tile_pool(name="psp", bufs=3, space="PSUM"))
        pvp = ctx.enter_context(tc.tile_pool(name="pvp", bufs=2, space="PSUM"))
        denp = ctx.enter_context(tc.tile_pool(name="denp", bufs=1, space="PSUM"))
        finp = ctx.enter_context(tc.tile_pool(name="finp", bufs=2, space="PSUM"))

        # ---- constants ----
        ones_col = const.tile([128, 1], f16, name="ones_col")
        nc.vector.memset(ones_col, 1.0)
        ones_row = const.tile([1, 128], f32, name="ones_row")
        nc.vector.memset(ones_row, 1.0)
        ind_fwd = const.tile([128, 4], f32, name="ind_fwd")   # [ch, grp] one-hot
        nc.vector.memset(ind_fwd, 0.0)
        for g in range(4):
            nc.vector.memset(ind_fwd[g * GS:(g + 1) * GS, g:g + 1], 1.0)
        ind_bwd = const.tile([4, 128], f32, name="ind_bwd")   # [grp, ch] one-hot
        nc.vector.memset(ind_bwd[0:4, :], 0.0)
        for g in range(4):
            # engines can't start at partition g∉{0,32,64,96}; DMA can
            nc.sync.dma_start(out=ind_bwd[g:g + 1, g * GS:(g + 1) * GS],
                              in_=ones_row[0:1, 0:GS])
        eps_t = const.tile([128, 1], f32, name="eps_t")
        nc.vector.memset(eps_t, EPS)

        # ---- weights / biases ----
        wt, owt, nw, nb, ob = [], [], [], [], []
        for ci in range(2):
            w = const.tile([128, 3 * C], f32, name=f"wt{ci}")
            nc.sync.dma_start(out=w, in_=qkv_wT[ci * 128:(ci + 1) * 128, :])
            wt.append(w)
            o = const.tile([128, C], f32, name=f"owt{ci}")
            nc.sync.dma_start(out=o, in_=out_wT[ci * 128:(ci + 1) * 128, :])
            owt.append(o)
            t = const.tile([128, 1], f32, name=f"nw{ci}")
            nc.sync.dma_start(out=t, in_=norm_w[ci * 128:(ci + 1) * 128, :])
            nw.append(t)
            t = const.tile([128, 1], f32, name=f"nb{ci}")
            nc.sync.dma_start(out=t, in_=norm_b[ci * 128:(ci + 1) * 128, :])
            nb.append(t)
            t = const.tile([128, 1], f32, name=f"ob{ci}")
            nc.sync.dma_start(out=t, in_=out_b[ci * 128:(ci + 1) * 128, :])
            ob.append(t)
        qb = []
        for j in range(6):
            t = const.tile([128, 1], f32, name=f"qb{j}")
            nc.sync.dma_start(out=t, in_=qkv_b[j * 128:(j + 1) * 128, :])
            qb.append(t)

        # fp16 casts of matmul weights (fp32 on PE = 2 HW passes at half rate)
        wt16, owt16 = [], []
        for ci in range(2):
            w16 = const.tile([128, 3 * C], f16, name=f"wt16_{ci}")
            nc.vector.tensor_copy(out=w16, in_=wt[ci])
            wt16.append(w16)
            o16 = const.tile([128, C], f16, name=f"owt16_{ci}")
            nc.vector.tensor_copy(out=o16, in_=owt[ci])
            owt16.append(o16)
        qb16v = []
        for j in (4, 5):
            t16 = const.tile([128, 1], f16, name=f"qb16_{j}")
            nc.vector.tensor_copy(out=t16, in_=qb[j])
            qb16v.append(t16)

        # ---- load x in [128,1024] chunks; stats pipelined per chunk ----
        xq = [[None] * 4 for _ in range(2)]
        for j in range(4):
            for ct in range(2):
                t = xqp.tile([128, 1024], f32, tag="xq", name=f"xq{ct}_{j}")
                nc.sync.dma_start(
                    out=t,
                    in_=x_kv[ct * 128:(ct + 1) * 128, j * 1024:(j + 1) * 1024])
                xq[ct][j] = t

        stats = []
        for ct in range(2):
            st = smal.tile([128, 8, 6], f32, tag="stats", name=f"stats{ct}")
            stats.append(st)
        for j in range(4):
            for ct in range(2):
                for s in range(2):
                    nc.vector.bn_stats(
                        out=stats[ct][:, j * 2 + s, :],
                        in_=xq[ct][j][:, s * 512:(s + 1) * 512])

        # group stats -> per-channel affine (A, B)
        AB = []
        for ct in range(2):
            mv = smal.tile([128, 2], f32, tag="mv", name=f"mv{ct}")
            nc.vector.bn_aggr(out=mv, in_=stats[ct])
            st2 = smal.tile([128, 2], f32, tag="st2", name=f"st2{ct}")
            nc.vector.tensor_copy(out=st2[:, 0:1], in_=mv[:, 0:1])
            nc.vector.tensor_mul(out=st2[:, 1:2], in0=mv[:, 0:1], in1=mv[:, 0:1])
            nc.vector.tensor_add(out=st2[:, 1:2], in0=st2[:, 1:2], in1=mv[:, 1:2])
            gp = psp.tile([4, 2], f32, tag="ps", name=f"gp{ct}")
            nc.tensor.matmul(gp, ind_fwd, st2, start=True, stop=True)
            gsb = smal.tile([4, 2], f32, tag="gsb", name=f"gsb{ct}")
            nc.scalar.mul(out=gsb, in_=gp, mul=1.0 / GS)
            chp = psp.tile([128, 2], f32, tag="ps", name=f"chp{ct}")
            nc.tensor.matmul(chp, ind_bwd, gsb, start=True, stop=True)
            ch = smal.tile([128, 2], f32, tag="ch", name=f"ch{ct}")
            nc.vector.tensor_copy(out=ch, in_=chp)
            var = smal.tile([128, 1], f32, tag="var", name=f"var{ct}")
            nc.vector.tensor_mul(out=var, in0=ch[:, 0:1], in1=ch[:, 0:1])
            nc.vector.tensor_sub(out=var, in0=ch[:, 1:2], in1=var)
            std = smal.tile([128, 1], f32, tag="std", name=f"std{ct}")
            nc.scalar.activation(out=std, in_=var, func=Act.Sqrt, bias=eps_t)
            rstd = smal.tile([128, 1], f32, tag="rstd", name=f"rstd{ct}")
            nc.vector.reciprocal(out=rstd, in_=std)
            A = smal.tile([128, 1], f32, tag="A", name=f"A{ct}")
            nc.vector.tensor_mul(out=A, in0=nw[ct], in1=rstd)
            Bb = smal.tile([128, 1], f32, tag="B", name=f"B{ct}")
            nc.vector.tensor_mul(out=Bb, in0=ch[:, 0:1], in1=A)
            nc.vector.tensor_sub(out=Bb, in0=nb[ct], in1=Bb)
            AB.append((A, Bb))

        # apply affine -> fp16 h chunks (ct0 on DVE, ct1 on ACT, in parallel)
        h16 = [[None] * 4 for _ in range(2)]
        for j in range(4):
            for ct in range(2):
                ht = hp.tile([128, 1024], f16, tag="h16", name=f"h16_{ct}_{j}")
                if ct == 0:
                    nc.vector.tensor_scalar(
                        out=ht, in0=xq[ct][j],
                        scalar1=AB[ct][0], scalar2=AB[ct][1],
                        op0=Alu.mult, op1=Alu.add)
                else:
                    nc.scalar.activation(
                        out=ht, in_=xq[ct][j], func=Act.Identity,
                        bias=AB[ct][1], scale=AB[ct][0])
                h16[ct][j] = ht

        def hsl(ct, nch):  # [128,512] slice of h16 for 512-col chunk nch
            return h16[ct][nch // 2][:, (nch % 2) * 512:((nch % 2) + 1) * 512]

        def hmt(ct, mt):   # [128,128] slice of h16 for 128-col tile mt
            return h16[ct][mt // 8][:, (mt % 8) * 128:((mt % 8) + 1) * 128]

        # ---- q conv (queries = first NQ cols) ----
        q_sb = [qp.tile([128, NQ], f16, tag="q", name=f"q{co}") for co in range(2)]
        for co in range(2):
            for nch in range(NQ // 512):
                pq = psp.tile([128, 512], f32, tag="ps", name=f"pq{co}_{nch}")
                nc.tensor.matmul(pq, wt16[0][:, co * 128:(co + 1) * 128],
                                 hsl(0, nch), start=True, stop=False)
                nc.tensor.matmul(pq, wt16[1][:, co * 128:(co + 1) * 128],
                                 hsl(1, nch), start=False, stop=True)
                nc.vector.tensor_scalar_add(
                    out=q_sb[co][:, nch * 512:(nch + 1) * 512],
                    in0=pq, scalar1=qb[co])

        # ---- k conv ----
        k_sb = [kp.tile([128, N], f16, tag="k", name=f"k{co}") for co in range(2)]
        for co in range(2):
            for nch in range(N // 512):
                pk = psp.tile([128, 512], f32, tag="ps", name=f"pk{co}_{nch}")
                nc.tensor.matmul(pk, wt16[0][:, 256 + co * 128:256 + (co + 1) * 128],
                                 hsl(0, nch), start=True, stop=False)
                nc.tensor.matmul(pk, wt16[1][:, 256 + co * 128:256 + (co + 1) * 128],
                                 hsl(1, nch), start=False, stop=True)
                nc.vector.tensor_scalar_add(
                    out=k_sb[co][:, nch * 512:(nch + 1) * 512],
                    in0=pk, scalar1=qb[2 + co])

        # ---- vT conv (no bias; folded into effb) ----
        vt_sb = []
        for mt in range(32):
            pvt = psp.tile([128, 256], f32, tag="ps", name=f"pvt{mt}")
            nc.tensor.matmul(pvt, hmt(0, mt), wt16[0][:, 512:768],
                             start=True, stop=False)
            nc.tensor.matmul(pvt, hmt(1, mt), wt16[1][:, 512:768],
                             start=False, stop=True)
            vt = vtp.tile([128, 256], f16, tag="vt", name=f"vt{mt}")
            nc.vector.tensor_copy(out=vt, in_=pvt)
            vt_sb.append(vt)

        # ---- effective output bias: out_w @ v_bias + out_b ----
        effb = []
        for co in range(2):
            pe_ = psp.tile([128, 1], f32, tag="ps", name=f"peffb{co}")
            nc.tensor.matmul(pe_, owt16[0][:, co * 128:(co + 1) * 128], qb16v[0],
                             start=True, stop=False)
            nc.tensor.matmul(pe_, owt16[1][:, co * 128:(co + 1) * 128], qb16v[1],
                             start=False, stop=True)
            e = smal.tile([128, 1], f32, tag="effb", name=f"effb{co}")
            nc.vector.tensor_add(out=e, in0=pe_, in1=ob[co])
            effb.append(e)

        # ---- attention ----
        def st_chunk(t, mch):
            ps_s = psp.tile([128, 512], f32, tag="ps", name=f"s_{t}_{mch}")
            nc.tensor.matmul(ps_s, k_sb[0][:, mch * 128:(mch + 1) * 128],
                             q_sb[0][:, t * 512:(t + 1) * 512],
                             start=True, stop=False)
            nc.tensor.matmul(ps_s, k_sb[1][:, mch * 128:(mch + 1) * 128],
                             q_sb[1][:, t * 512:(t + 1) * 512],
                             start=False, stop=True)
            es = esp.tile([128, 512], f16, tag="es", name=f"es_{t}_{mch}")
            nc.scalar.activation(out=es, in_=ps_s, func=Act.Exp, scale=SCALE)
            return es

        def epilogue_head(t, pv, den):
            # 1/den is off the PE critical path: normalization commutes
            # through the output projection as a per-query column scale
            dsb = smal.tile([1, 512], f32, tag="dsb", name=f"dsb_{t}")
            nc.vector.tensor_copy(out=dsb, in_=den)
            rdsb = smal.tile([1, 512], f32, tag="rdsb", name=f"rdsb_{t}")
            nc.vector.reciprocal(out=rdsb, in_=dsb)
            pvsb = []
            for co in range(2):
                p = sb2.tile([128, 512], f16, tag=f"pvsb{co}",
                             name=f"pvsb_{t}_{co}")
                nc.scalar.copy(out=p, in_=pv[co])     # frees the pv psum bank
                pvsb.append(p)
            fins = []
            for co in range(2):
                fin = finp.tile([128, 512], f32, tag="fin", name=f"fin_{t}_{co}")
                nc.tensor.matmul(fin, owt16[0][:, co * 128:(co + 1) * 128],
                                 pvsb[0], start=True, stop=False)
                nc.tensor.matmul(fin, owt16[1][:, co * 128:(co + 1) * 128],
                                 pvsb[1], start=False, stop=True)
                fins.append(fin)
            return rdsb, fins

        def epilogue_tail(t, rdsb, fins):
            bc = psp.tile([128, 512], f32, tag="ps", name=f"bc_{t}")
            nc.tensor.matmul(bc, ones_row, rdsb, start=True, stop=True)
            rdb = sb2.tile([128, 512], f32, tag="rdb", name=f"rdb_{t}")
            nc.scalar.copy(out=rdb, in_=bc)
            for co in range(2):
                rsb = sb2.tile([128, 512], f32, tag=f"rsb{co}", name=f"r_{t}_{co}")
                nc.sync.dma_start(
                    out=rsb,
                    in_=x_kv[co * 128:(co + 1) * 128, t * 512:(t + 1) * 512])
                t1 = sb2.tile([128, 512], f32, tag=f"t1{co}", name=f"t1_{t}_{co}")
                nc.vector.tensor_mul(out=t1, in0=fins[co], in1=rdb)
                ysb = sb2.tile([128, 512], f32, tag=f"ysb{co}", name=f"y_{t}_{co}")
                nc.vector.scalar_tensor_tensor(
                    out=ysb, in0=t1, scalar=effb[co], in1=rsb,
                    op0=Alu.add, op1=Alu.add)
                nc.sync.dma_start(
                    out=y[co * 128:(co + 1) * 128, t * 512:(t + 1) * 512],
                    in_=ysb)

        NT = NQ // 512
        pending = None  # (t, rdsb, fins) whose tail is deferred
        for t in range(NT):
            pv = [pvp.tile([128, 512], f32, tag="pv", name=f"pv_{t}_{co}")
                  for co in range(2)]
            den = denp.tile([1, 512], f32, tag="den", name=f"den_{t}")
            es_q = {0: st_chunk(t, 0), 1: st_chunk(t, 1)}
            for mch in range(32):
                es = es_q.pop(mch)
                st, sp = (mch == 0), (mch == 31)
                nc.tensor.matmul(pv[0], vt_sb[mch][:, 0:128], es, start=st, stop=sp)
                nc.tensor.matmul(pv[1], vt_sb[mch][:, 128:256], es, start=st, stop=sp)
                nc.tensor.matmul(den, ones_col, es, start=st, stop=sp)
                if mch + 2 < 32:
                    es_q[mch + 2] = st_chunk(t, mch + 2)
                if mch == 6 and pending is not None:
                    epilogue_tail(*pending)   # deferred so PE hides the recip
                    pending = None
            rdsb, fins = epilogue_head(t, pv, den)
            pending = (t, rdsb, fins)
        epilogue_tail(*pending)

    _split_multiwait_instructions(nc)
    return nc


def _get_nc():
    if "nc" not in _CACHE:
        _CACHE["nc"] = _build()
    return _CACHE["nc"]


def kernel(x, norm_w, norm_b, qkv_w, qkv_b, out_w, out_b):
    from concourse.bass_utils import run_bass_kernel_spmd

    nc = _get_nc()
    xr = np.ascontiguousarray(np.asarray(x, dtype=np.float32).reshape(B, C, N))
    wT = np.ascontiguousarray(np.asarray(qkv_w, np.float32).T)
    owT = np.ascontiguousarray(np.asarray(out_w, np.float32).T)
    qb2 = np.ascontiguousarray(np.asarray(qkv_b, np.float32).reshape(3 * C, 1))
    nw2 = np.ascontiguousarray(np.asarray(norm_w, np.float32).reshape(C, 1))
    nb2 = np.ascontiguousarray(np.asarray(norm_b, np.float32).reshape(C, 1))
    ob2 = np.ascontiguousarray(np.asarray(out_b, np.float32).reshape(C, 1))

    in_maps = []
    for c in range(8):
        b, h = divmod(c, 2)
        off = h * NQ
        xb = xr[b]
        if off:
            xroll = np.ascontiguousarray(
                np.concatenate([xb[:, off:], xb[:, :off]], axis=1))
        else:
            xroll = xb
        in_maps.append({
            "x_kv": xroll, "qkv_wT": wT, "qkv_b": qb2,
            "norm_w": nw2, "norm_b": nb2, "out_wT": owT, "out_b": ob2,
        })

    res = run_bass_kernel_spmd(nc, in_maps, core_ids=list(range(8)))
    out = np.empty((B, C, N), np.float32)
    for c in range(8):
        b, h = divmod(c, 2)
        off = h * NQ
        out[b][:, off:off + NQ] = res.results[c]["y"]
    return out.reshape(B, C, 16, 16, 16)


# revision 14
# speedup vs baseline: 3.0234x; 1.0194x over previous
"""AttentionBlock3D (GroupNorm + single-head attention over 4096 tokens + residual)
on 8 Trainium2 NeuronCores.

Sharding: core c handles sample b=c//2 and query half h=c%2 (2048 queries).
The host rotates each sample's token axis so that each core's queries are
columns 0..2048 of its x_kv input (attention is permutation-invariant over
keys, and groupnorm stats are permutation-invariant), letting a single SPMD
program serve all 8 cores with no dynamic offsets.

Device-side math per core:
  h = groupnorm(x)                   fp32 stats (bn_stats + indicator-matmul
                                     cross-partition group reduce), fp16 output
  q = Wq h + bq  [256,2048] fp16     (host passes qkv_w^T; no on-chip transposes)
  k = Wk h + bk  [256,4096] fp16
  vT = (Wv h)^T  [4096,256] fp16     (computed directly in transposed layout)
  S^T tiles [128 keys, 512 queries] -> es = exp(S^T * C^-0.5) fp16
                                     (no max subtraction: |scores*scale| <~ 8)
  pv[c,q]  = sum_m vT[m,c] es[m,q]   (v chunks stationary, es streams, fp32 psum)
  den[q]   = sum_m es[m,q]           (ones-column matmul)
  fin      = out_w @ pv              (normalization deferred: a per-query column
                                      scale commutes through the projection)
  y = fin * (1/den bcast) + (out_w @ bv + ob) + x   (v-bias folded via sum(A)=1)

All matmuls run in fp16 (fp32 on the PE lowers to 2 HW passes at half rate);
PSUM accumulation is fp32 throughout, softmax statistics are fp32.
"""
import numpy as np

_CACHE: dict = {}

B, C, N = 4, 256, 4096
NQ = 2048          # queries per core
G = 8              # groups
GS = 32            # channels per group
EPS = 1e-5
SCALE = C ** -0.5


def _install_tile_drain_patch():
    """This container's walrus only supports ONE sync-wait command per
    instruction; TileContext's final drain carries one wait per live proc and
    fails codegen. Re-emit the waits as standalone sem-wait instructions."""
    import concourse.tile as tile_mod
    from concourse.vector_clock import ScopedClock

    if getattr(tile_mod.TileContext, "_ant_drain_patched", False):
        return

    def _patched(self, tick_clock, wait_clock):
        nc = self.nc
        probe = nc.sync.nop(nofuse=True)
        wait_clock.add_sem_waits(
            probe.ins, ScopedClock({None: tick_clock.global_clock})
        )
        waits = list(probe.ins.sync_info.on_wait or [])
        if len(waits) > 1:
            probe.ins.sync_info.on_wait = []
            num2h = {}
            assert self.sems is not None
            for h in self.sems.allocated().values():
                num2h[h.num] = h
            for h in self.sems.swdge_block():
                num2h.setdefault(h.num, h)
            for w in waits:
                h = num2h.get(w.id)
                assert h is not None, f"sem id {w.id} ({w.ant_name}) has no handle"
                nc.sync.wait_op(h, w.wait_value, "sem-ge")
        nc.sync.drain()
        nc.all_engine_barrier()
        assert self.sems is not None
        popped = nc._tile_sem_poison_stack.pop()
        assert popped is self._sem_poison
        nc.clear_and_free_semaphores(list(self.sems.allocated().values()))
        nc.all_engine_barrier()

    tile_mod.TileContext._drain_and_barrier = _patched
    tile_mod.TileContext._ant_drain_patched = True


def _split_multiwait_instructions(nc):
    """This walrus build supports only one sync-wait command per instruction.
    Hoist extra waits into standalone InstEventSemaphore waits on the same
    engine immediately before the instruction (sequential single waits are
    equivalent to one multi-wait)."""
    import concourse.mybir as mybir

    n = 0
    for f in nc.m.functions:
        for bb in f.blocks:
            new = []
            for ins in bb.instructions:
                si = ins.sync_info
                waits = list(si.on_wait) if si else []
                if len(waits) > 1:
                    for w in waits[:-1]:
                        nop = mybir.InstEventSemaphore(
                            name=f"WSPLIT-{n}", ins=[], outs=[])
                        n += 1
                        nop.engine = ins.engine
                        nop.sync_info = mybir.SyncInfo(on_wait=[w], on_update=[])
                        new.append(nop)
                    si.on_wait = [waits[-1]]
                new.append(ins)
            bb.instructions[:] = new


def _build():
    import concourse.bass as bass
    import concourse.tile as tile
    import concourse.mybir as mybir
    from contextlib import ExitStack

    _install_tile_drain_patch()

    f32 = mybir.dt.float32
    f16 = mybir.dt.float16
    Act = mybir.ActivationFunctionType
    Alu = mybir.AluOpType

    nc = bass.Bass()
    x_kv = nc.dram_tensor("x_kv", [C, N], f32, kind="ExternalInput")
    # w_pack = [qkv_wT | out_wT] : [256, 1024]; vec_pack cols =
    # qb0..qb5, nw0, nw1, nb0, nb1, ob0, ob1 : [128, 12]
    w_pack = nc.dram_tensor("w_pack", [C, 4 * C], f32, kind="ExternalInput")
    vec_pack = nc.dram_tensor("vec_pack", [128, 12], f32, kind="ExternalInput")
    y = nc.dram_tensor("y", [C, NQ], f32, kind="ExternalOutput")

    with ExitStack() as ctx:
        tc = ctx.enter_context(tile.TileContext(nc))
        const = ctx.enter_context(tc.tile_pool(name="const", bufs=1))
        xqp = ctx.enter_context(tc.tile_pool(name="xqp", bufs=8))
        hp = ctx.enter_context(tc.tile_pool(name="hp", bufs=8))
        kp = ctx.enter_context(tc.tile_pool(name="kp", bufs=2))
        qp = ctx.enter_context(tc.tile_pool(name="qp", bufs=2))
        vtp = ctx.enter_context(tc.tile_pool(name="vtp", bufs=32))
        esp = ctx.enter_context(tc.tile_pool(name="esp", bufs=16))
        smal = ctx.enter_context(tc.tile_pool(name="smal", bufs=2))
        sb2 = ctx.enter_context(tc.tile_pool(name="sb2", bufs=2))
        psp = ctx.enter_context(tc.tile_pool(name="psp", bufs=3, space="PSUM"))
        pvp = ctx.enter_context(tc.tile_pool(name="pvp", bufs=2, space="PSUM"))
        denp = ctx.enter_context(tc.tile_pool(name="denp", bufs=1, space="PSUM"))
        finp = ctx.enter_context(tc.tile_pool(name="finp", bufs=2, space="PSUM"))

        # ---- constants ----
        ones_col = const.tile([128, 1], f16, name="ones_col")
        nc.vector.memset(ones_col, 1.0)
        ones_row = const.tile([1, 128], f32, name="ones_row")
        nc.vector.memset(ones_row, 1.0)
        ind_fwd = const.tile([128, 4], f32, name="ind_fwd")   # [ch, grp] one-hot
        nc.vector.memset(ind_fwd, 0.0)
        for g in range(4):
            nc.vector.memset(ind_fwd[g * GS:(g + 1) * GS, g:g + 1], 1.0)
        ind_bwd = const.tile([4, 128], f32, name="ind_bwd")   # [grp, ch] one-hot
        nc.vector.memset(ind_bwd[0:4, :], 0.0)
        for g in range(4):
            # engines can't start at partition g∉{0,32,64,96}; DMA can
            nc.sync.dma_start(out=ind_bwd[g:g + 1, g * GS:(g + 1) * GS],
                              in_=ones_row[0:1, 0:GS])
        eps_t = const.tile([128, 1], f32, name="eps_t")
        nc.vector.memset(eps_t, EPS)

        # ---- load x FIRST in [128,1024] chunks, split across two DMA
        #      engines (each dma_start costs ~600ns of issue time serially
        #      per engine) ----
        xq = [[None] * 4 for _ in range(2)]
        for j in range(4):
            for ct in range(2):
                t = xqp.tile([128, 1024], f32, tag="xq", name=f"xq{ct}_{j}")
                eng = nc.sync if ct == 0 else nc.gpsimd
                eng.dma_start(
                    out=t,
                    in_=x_kv[ct * 128:(ct + 1) * 128, j * 1024:(j + 1) * 1024])
                xq[ct][j] = t

        # ---- packed weights / biases (2 + 1 DMA issues) ----
        wpk = []
        for ci in range(2):
            w = const.tile([128, 4 * C], f32, name=f"wpk{ci}")
            nc.sync.dma_start(out=w, in_=w_pack[ci * 128:(ci + 1) * 128, :])
            wpk.append(w)
        vp = const.tile([128, 12], f32, name="vp")
        nc.gpsimd.dma_start(out=vp, in_=vec_pack[:, :])
        wt = [wpk[ci][:, 0:3 * C] for ci in range(2)]
        owt = [wpk[ci][:, 3 * C:4 * C] for ci in range(2)]
        qb = [vp[:, j:j + 1] for j in range(6)]
        nw = [vp[:, 6 + ci:7 + ci] for ci in range(2)]
        nb = [vp[:, 8 + ci:9 + ci] for ci in range(2)]
        ob = [vp[:, 10 + ci:11 + ci] for ci in range(2)]

        # fp16 casts of matmul weights (fp32 on PE = 2 HW passes at half rate)
        wt16, owt16 = [], []
        for ci in range(2):
            w16 = const.tile([128, 3 * C], f16, name=f"wt16_{ci}")
            nc.vector.tensor_copy(out=w16, in_=wt[ci])
            wt16.append(w16)
            o16 = const.tile([128, C], f16, name=f"owt16_{ci}")
            nc.vector.tensor_copy(out=o16, in_=owt[ci])
            owt16.append(o16)
        qb16v = []
        for j in (4, 5):
            t16 = const.tile([128, 1], f16, name=f"qb16_{j}")
            nc.vector.tensor_copy(out=t16, in_=qb[j])
            qb16v.append(t16)

        # ct0: DVE bn_stats (Welford); ct1: ACT accumulate sum & sumsq
        # per chunk -- the two engines run in parallel.
        stats0 = smal.tile([128, 8, 6], f32, tag="stats", name="stats0")
        for j in range(4):
            for s in range(2):
                nc.vector.bn_stats(
                    out=stats0[:, j * 2 + s, :],
                    in_=xq[0][j][:, s * 512:(s + 1) * 512])
        sc_scr = smal.tile([128, 1024], f32, tag="sc_scr", name="sc_scr")
        acc1 = smal.tile([128, 8], f32, tag="acc1", name="acc1")  # sums|sqsums
        for j in range(4):
            nc.scalar.activation(out=sc_scr, in_=xq[1][j], func=Act.Identity,
                                 accum_out=acc1[:, j:j + 1])
            nc.scalar.activation(out=sc_scr, in_=xq[1][j], func=Act.Square,
                                 accum_out=acc1[:, 4 + j:5 + j])

        # group stats -> per-channel affine (A, B)
        AB = []
        for ct in range(2):
            st2 = smal.tile([128, 2], f32, tag="st2", name=f"st2{ct}")
            if ct == 0:
                mv = smal.tile([128, 2], f32, tag="mv", name=f"mv{ct}")
                nc.vector.bn_aggr(out=mv, in_=stats0)
                nc.vector.tensor_copy(out=st2[:, 0:1], in_=mv[:, 0:1])
                nc.vector.tensor_mul(out=st2[:, 1:2], in0=mv[:, 0:1], in1=mv[:, 0:1])
                nc.vector.tensor_add(out=st2[:, 1:2], in0=st2[:, 1:2], in1=mv[:, 1:2])
            else:
                # st2 = [mean, E[x^2]] from the 4+4 partial sums
                pa = smal.tile([128, 4], f32, tag="pa", name="pa1")
                nc.vector.tensor_add(out=pa[:, 0:2], in0=acc1[:, 0:2],
                                     in1=acc1[:, 2:4])
                nc.vector.tensor_add(out=pa[:, 2:4], in0=acc1[:, 4:6],
                                     in1=acc1[:, 6:8])
                pb = smal.tile([128, 2], f32, tag="pb", name="pb1")
                nc.vector.tensor_add(out=pb[:, 0:1], in0=pa[:, 0:1], in1=pa[:, 1:2])
                nc.vector.tensor_add(out=pb[:, 1:2], in0=pa[:, 2:3], in1=pa[:, 3:4])
                nc.vector.tensor_scalar_mul(out=st2, in0=pb, scalar1=1.0 / N)
            gp = psp.tile([4, 2], f32, tag="ps", name=f"gp{ct}")
            nc.tensor.matmul(gp, ind_fwd, st2, start=True, stop=True)
            gsb = smal.tile([4, 2], f32, tag="gsb", name=f"gsb{ct}")
            nc.scalar.mul(out=gsb, in_=gp, mul=1.0 / GS)
            chp = psp.tile([128, 2], f32, tag="ps", name=f"chp{ct}")
            nc.tensor.matmul(chp, ind_bwd, gsb, start=True, stop=True)
            ch = smal.tile([128, 2], f32, tag="ch", name=f"ch{ct}")
            nc.vector.tensor_copy(out=ch, in_=chp)
            var = smal.tile([128, 1], f32, tag="var", name=f"var{ct}")
            nc.vector.tensor_mul(out=var, in0=ch[:, 0:1], in1=ch[:, 0:1])
            nc.vector.tensor_sub(out=var, in0=ch[:, 1:2], in1=var)
            std = smal.tile([128, 1], f32, tag="std", name=f"std{ct}")
            nc.scalar.activation(out=std, in_=var, func=Act.Sqrt, bias=eps_t)
            rstd = smal.tile([128, 1], f32, tag="rstd", name=f"rstd{ct}")
            nc.vector.reciprocal(out=rstd, in_=std)
            A = smal.tile([128, 1], f32, tag="A", name=f"A{ct}")
            nc.vector.tensor_mul(out=A, in0=nw[ct], in1=rstd)
            Bb = smal.tile([128, 1], f32, tag="B", name=f"B{ct}")
            nc.vector.tensor_mul(out=Bb, in0=ch[:, 0:1], in1=A)
            nc.vector.tensor_sub(out=Bb, in0=nb[ct], in1=Bb)
            AB.append((A, Bb))

        # apply affine -> fp16 h chunks (ct0 on DVE, ct1 on ACT, in parallel)
        h16 = [[None] * 4 for _ in range(2)]
        for j in range(4):
            for ct in range(2):
                ht = hp.tile([128, 1024], f16, tag="h16", name=f"h16_{ct}_{j}")
                if ct == 0:
                    nc.vector.tensor_scalar(
                        out=ht, in0=xq[ct][j],
                        scalar1=AB[ct][0], scalar2=AB[ct][1],
                        op0=Alu.mult, op1=Alu.add)
                else:
                    nc.scalar.activation(
                        out=ht, in_=xq[ct][j], func=Act.Identity,
                        bias=AB[ct][1], scale=AB[ct][0])
                h16[ct][j] = ht

        def hsl(ct, nch):  # [128,512] slice of h16 for 512-col chunk nch
            return h16[ct][nch // 2][:, (nch % 2) * 512:((nch % 2) + 1) * 512]

        def hmt(ct, mt):   # [128,128] slice of h16 for 128-col tile mt
            return h16[ct][mt // 8][:, (mt % 8) * 128:((mt % 8) + 1) * 128]

        # ---- q conv (queries = first NQ cols) ----
        q_sb = [qp.tile([128, NQ], f16, tag="q", name=f"q{co}") for co in range(2)]
        for co in range(2):
            for nch in range(NQ // 512):
                pq = psp.tile([128, 512], f32, tag="ps", name=f"pq{co}_{nch}")
                nc.tensor.matmul(pq, wt16[0][:, co * 128:(co + 1) * 128],
                                 hsl(0, nch), start=True, stop=False)
                nc.tensor.matmul(pq, wt16[1][:, co * 128:(co + 1) * 128],
                                 hsl(1, nch), start=False, stop=True)
                nc.vector.tensor_scalar_add(
                    out=q_sb[co][:, nch * 512:(nch + 1) * 512],
                    in0=pq, scalar1=qb[co])

        # ---- k conv ----
        k_sb = [kp.tile([128, N], f16, tag="k", name=f"k{co}") for co in range(2)]
        for co in range(2):
            for nch in range(N // 512):
                pk = psp.tile([128, 512], f32, tag="ps", name=f"pk{co}_{nch}")
                nc.tensor.matmul(pk, wt16[0][:, 256 + co * 128:256 + (co + 1) * 128],
                                 hsl(0, nch), start=True, stop=False)
                nc.tensor.matmul(pk, wt16[1][:, 256 + co * 128:256 + (co + 1) * 128],
                                 hsl(1, nch), start=False, stop=True)
                nc.vector.tensor_scalar_add(
                    out=k_sb[co][:, nch * 512:(nch + 1) * 512],
                    in0=pk, scalar1=qb[2 + co])

        # ---- vT conv (no bias; folded into effb) ----
        vt_sb = []
        for mt in range(32):
            pvt = psp.tile([128, 256], f32, tag="ps", name=f"pvt{mt}")
            nc.tensor.matmul(pvt, hmt(0, mt), wt16[0][:, 512:768],
                             start=True, stop=False)
            nc.tensor.matmul(pvt, hmt(1, mt), wt16[1][:, 512:768],
                             start=False, stop=True)
            vt = vtp.tile([128, 256], f16, tag="vt", name=f"vt{mt}")
            nc.vector.tensor_copy(out=vt, in_=pvt)
            vt_sb.append(vt)

        # ---- effective output bias: out_w @ v_bias + out_b ----
        effb = []
        for co in range(2):
            pe_ = psp.tile([128, 1], f32, tag="ps", name=f"peffb{co}")
            nc.tensor.matmul(pe_, owt16[0][:, co * 128:(co + 1) * 128], qb16v[0],
                             start=True, stop=False)
            nc.tensor.matmul(pe_, owt16[1][:, co * 128:(co + 1) * 128], qb16v[1],
                             start=False, stop=True)
            e = smal.tile([128, 1], f32, tag="effb", name=f"effb{co}")
            nc.vector.tensor_add(out=e, in0=pe_, in1=ob[co])
            effb.append(e)

        # ---- attention ----
        def st_chunk(t, mch):
            ps_s = psp.tile([128, 512], f32, tag="ps", name=f"s_{t}_{mch}")
            nc.tensor.matmul(ps_s, k_sb[0][:, mch * 128:(mch + 1) * 128],
                             q_sb[0][:, t * 512:(t + 1) * 512],
                             start=True, stop=False)
            nc.tensor.matmul(ps_s, k_sb[1][:, mch * 128:(mch + 1) * 128],
                             q_sb[1][:, t * 512:(t + 1) * 512],
                             start=False, stop=True)
            es = esp.tile([128, 512], f16, tag="es", name=f"es_{t}_{mch}")
            nc.scalar.activation(out=es, in_=ps_s, func=Act.Exp, scale=SCALE)
            return es

        def epilogue_head(t, pv, den):
            # 1/den is off the PE critical path: normalization commutes
            # through the output projection as a per-query column scale
            dsb = smal.tile([1, 512], f32, tag="dsb", name=f"dsb_{t}")
            nc.vector.tensor_copy(out=dsb, in_=den)
            rdsb = smal.tile([1, 512], f32, tag="rdsb", name=f"rdsb_{t}")
            nc.vector.reciprocal(out=rdsb, in_=dsb)
            pvsb = []
            for co in range(2):
                p = sb2.tile([128, 512], f16, tag=f"pvsb{co}",
                             name=f"pvsb_{t}_{co}")
                nc.vector.tensor_copy(out=p, in_=pv[co])  # frees the pv psum bank
                pvsb.append(p)
            fins = []
            for co in range(2):
                fin = finp.tile([128, 512], f32, tag="fin", name=f"fin_{t}_{co}")
                nc.tensor.matmul(fin, owt16[0][:, co * 128:(co + 1) * 128],
                                 pvsb[0], start=True, stop=False)
                nc.tensor.matmul(fin, owt16[1][:, co * 128:(co + 1) * 128],
                                 pvsb[1], start=False, stop=True)
                fins.append(fin)
            return rdsb, fins

        def epilogue_tail(t, rdsb, fins):
            bc = psp.tile([128, 512], f32, tag="ps", name=f"bc_{t}")
            nc.tensor.matmul(bc, ones_row, rdsb, start=True, stop=True)
            rdb = sb2.tile([128, 512], f32, tag="rdb", name=f"rdb_{t}")
            nc.vector.tensor_copy(out=rdb, in_=bc)
            for co in range(2):
                rsb = sb2.tile([128, 512], f32, tag=f"rsb{co}", name=f"r_{t}_{co}")
                nc.sync.dma_start(
                    out=rsb,
                    in_=x_kv[co * 128:(co + 1) * 128, t * 512:(t + 1) * 512])
                t1 = sb2.tile([128, 512], f32, tag=f"t1{co}", name=f"t1_{t}_{co}")
                nc.vector.tensor_mul(out=t1, in0=fins[co], in1=rdb)
                ysb = sb2.tile([128, 512], f32, tag=f"ysb{co}", name=f"y_{t}_{co}")
                nc.vector.scalar_tensor_tensor(
                    out=ysb, in0=t1, scalar=effb[co], in1=rsb,
                    op0=Alu.add, op1=Alu.add)
                nc.sync.dma_start(
                    out=y[co * 128:(co + 1) * 128, t * 512:(t + 1) * 512],
                    in_=ysb)

        NT = NQ // 512
        pending = None  # (t, rdsb, fins) whose tail is deferred
        for t in range(NT):
            pv = [pvp.tile([128, 512], f32, tag="pv", name=f"pv_{t}_{co}")
                  for co in range(2)]
            den = denp.tile([1, 512], f32, tag="den", name=f"den_{t}")
            es_q = {0: st_chunk(t, 0), 1: st_chunk(t, 1)}
            for mch in range(32):
                es = es_q.pop(mch)
                st, sp = (mch == 0), (mch == 31)
                nc.tensor.matmul(pv[0], vt_sb[mch][:, 0:128], es, start=st, stop=sp)
                nc.tensor.matmul(pv[1], vt_sb[mch][:, 128:256], es, start=st, stop=sp)
                nc.tensor.matmul(den, ones_col, es, start=st, stop=sp)
                if mch + 2 < 32:
                    es_q[mch + 2] = st_chunk(t, mch + 2)
                if mch == 6 and pending is not None:
                    epilogue_tail(*pending)   # deferred so PE hides the recip
                    pending = None
            rdsb, fins = epilogue_head(t, pv, den)
            pending = (t, rdsb, fins)
        epilogue_tail(*pending)

    _split_multiwait_instructions(nc)
    return nc


def _get_nc():
    if "nc" not in _CACHE:
        _CACHE["nc"] = _build()
    return _CACHE["nc"]


def _prep_in_maps(x, norm_w, norm_b, qkv_w, qkv_b, out_w, out_b):
    xr = np.ascontiguousarray(np.asarray(x, dtype=np.float32).reshape(B, C, N))
    wT = np.asarray(qkv_w, np.float32).T
    owT = np.asarray(out_w, np.float32).T
    w_pack = np.ascontiguousarray(np.concatenate([wT, owT], axis=1))
    qb1 = np.asarray(qkv_b, np.float32)
    vec_pack = np.ascontiguousarray(np.stack(
        [qb1[0:128], qb1[128:256], qb1[256:384], qb1[384:512],
         qb1[512:640], qb1[640:768],
         np.asarray(norm_w, np.float32)[0:128],
         np.asarray(norm_w, np.float32)[128:256],
         np.asarray(norm_b, np.float32)[0:128],
         np.asarray(norm_b, np.float32)[128:256],
         np.asarray(out_b, np.float32)[0:128],
         np.asarray(out_b, np.float32)[128:256]], axis=1))

    in_maps = []
    for c in range(8):
        b, h = divmod(c, 2)
        off = h * NQ
        xb = xr[b]
        if off:
            xroll = np.ascontiguousarray(
                np.concatenate([xb[:, off:], xb[:, :off]], axis=1))
        else:
            xroll = xb
        in_maps.append({"x_kv": xroll, "w_pack": w_pack, "vec_pack": vec_pack})
    return in_maps


def kernel(x, norm_w, norm_b, qkv_w, qkv_b, out_w, out_b):
    from concourse.bass_utils import run_bass_kernel_spmd

    nc = _get_nc()
    in_maps = _prep_in_maps(x, norm_w, norm_b, qkv_w, qkv_b, out_w, out_b)
    res = run_bass_kernel_spmd(nc, in_maps, core_ids=list(range(8)))
    out = np.empty((B, C, N), np.float32)
    for c in range(8):
        b, h = divmod(c, 2)
        off = h * NQ
        out[b][:, off:off + NQ] = res.results[c]["y"]
    return out.reshape(B, C, 16, 16, 16)


# revision 16
# speedup vs baseline: 3.4286x; 1.1340x over previous
"""AttentionBlock3D (GroupNorm + single-head attention over 4096 tokens + residual)
on 8 Trainium2 NeuronCores.

Sharding: core c handles sample b=c//2 and query half h=c%2 (2048 queries).
The host rotates each sample's token axis so that each core's queries are
columns 0..2048 of its x_kv input (attention is permutation-invariant over
keys, and groupnorm stats are permutation-invariant), letting a single SPMD
program serve all 8 cores with no dynamic offsets.

Device-side math per core:
  h = groupnorm(x)                   fp32 stats (bn_stats + indicator-matmul
                                     cross-partition group reduce), fp16 output
  q = Wq h + bq  [256,2048] fp16     (host passes qkv_w^T; no on-chip transposes)
  k = Wk h + bk  [256,4096] fp16
  vT = (Wv h)^T  [4096,256] fp16     (computed directly in transposed layout)
  S^T tiles [128 keys, 512 queries] -> es = exp(S^T * C^-0.5) fp16
                                     (no max subtraction: |scores*scale| <~ 8)
  pv[c,q]  = sum_m vT[m,c] es[m,q]   (v chunks stationary, es streams, fp32 psum)
  den[q]   = sum_m es[m,q]           (ones-column matmul)
  fin      = out_w @ pv              (normalization deferred: a per-query column
                                      scale commutes through the projection)
  y = fin * (1/den bcast) + (out_w @ bv + ob) + x   (v-bias folded via sum(A)=1)

All matmuls run in fp16 (fp32 on the PE lowers to 2 HW passes at half rate);
PSUM accumulation is fp32 throughout, softmax statistics are fp32.
"""
import numpy as np

_CACHE: dict = {}

B, C, N = 4, 256, 4096
NQ = 2048          # queries per core
G = 8              # groups
GS = 32            # channels per group
EPS = 1e-5
SCALE = C ** -0.5


def _install_tile_drain_patch():
    """This container's walrus only supports ONE sync-wait command per
    instruction; TileContext's final drain carries one wait per live proc and
    fails codegen. Re-emit the waits as standalone sem-wait instructions."""
    import concourse.tile as tile_mod
    from concourse.vector_clock import ScopedClock

    if getattr(tile_mod.TileContext, "_ant_drain_patched", False):
        return

    def _patched(self, tick_clock, wait_clock):
        nc = self.nc
        probe = nc.sync.nop(nofuse=True)
        wait_clock.add_sem_waits(
            probe.ins, ScopedClock({None: tick_clock.global_clock})
        )
        waits = list(probe.ins.sync_info.on_wait or [])
        if len(waits) > 1:
            probe.ins.sync_info.on_wait = []
            num2h = {}
            assert self.sems is not None
            for h in self.sems.allocated().values():
                num2h[h.num] = h
            for h in self.sems.swdge_block():
                num2h.setdefault(h.num, h)
            for w in waits:
                h = num2h.get(w.id)
                assert h is not None, f"sem id {w.id} ({w.ant_name}) has no handle"
                nc.sync.wait_op(h, w.wait_value, "sem-ge")
        nc.sync.drain()
        nc.all_engine_barrier()
        assert self.sems is not None
        popped = nc._tile_sem_poison_stack.pop()
        assert popped is self._sem_poison
        nc.clear_and_free_semaphores(list(self.sems.allocated().values()))
        nc.all_engine_barrier()

    tile_mod.TileContext._drain_and_barrier = _patched
    tile_mod.TileContext._ant_drain_patched = True


def _split_multiwait_instructions(nc):
    """This walrus build supports only one sync-wait command per instruction.
    Hoist extra waits into standalone InstEventSemaphore waits on the same
    engine immediately before the instruction (sequential single waits are
    equivalent to one multi-wait)."""
    import concourse.mybir as mybir

    n = 0
    for f in nc.m.functions:
        for bb in f.blocks:
            new = []
            for ins in bb.instructions:
                si = ins.sync_info
                waits = list(si.on_wait) if si else []
                if len(waits) > 1:
                    for w in waits[:-1]:
                        nop = mybir.InstEventSemaphore(
                            name=f"WSPLIT-{n}", ins=[], outs=[])
                        n += 1
                        nop.engine = ins.engine
                        nop.sync_info = mybir.SyncInfo(on_wait=[w], on_update=[])
                        new.append(nop)
                    si.on_wait = [waits[-1]]
                new.append(ins)
            bb.instructions[:] = new


def _build():
    import concourse.bass as bass
    import concourse.tile as tile
    import concourse.mybir as mybir
    from contextlib import ExitStack

    _install_tile_drain_patch()

    f32 = mybir.dt.float32
    f16 = mybir.dt.float16
    Act = mybir.ActivationFunctionType
    Alu = mybir.AluOpType

    nc = bass.Bass()
    x_kv = nc.dram_tensor("x_kv", [C, N], f32, kind="ExternalInput")
    # w_pack = [qkv_wT | out_wT] : [256, 1024]; vec_pack cols =
    # qb0..qb5, nw0, nw1, nb0, nb1, ob0, ob1 : [128, 12]
    w_pack = nc.dram_tensor("w_pack", [C, 4 * C], f32, kind="ExternalInput")
    vec_pack = nc.dram_tensor("vec_pack", [128, 12], f32, kind="ExternalInput")
    y = nc.dram_tensor("y", [C, NQ], f32, kind="ExternalOutput")

    with ExitStack() as ctx:
        tc = ctx.enter_context(tile.TileContext(nc))
        const = ctx.enter_context(tc.tile_pool(name="const", bufs=1))
        xqp = ctx.enter_context(tc.tile_pool(name="xqp", bufs=8))
        hp = ctx.enter_context(tc.tile_pool(name="hp", bufs=8))
        kp = ctx.enter_context(tc.tile_pool(name="kp", bufs=2))
        qp = ctx.enter_context(tc.tile_pool(name="qp", bufs=2))
        vtp = ctx.enter_context(tc.tile_pool(name="vtp", bufs=32))
        esp = ctx.enter_context(tc.tile_pool(name="esp", bufs=16))
        smal = ctx.enter_context(tc.tile_pool(name="smal", bufs=2))
        sb2 = ctx.enter_context(tc.tile_pool(name="sb2", bufs=2))
        psp = ctx.enter_context(tc.tile_pool(name="psp", bufs=3, space="PSUM"))
        pvp = ctx.enter_context(tc.tile_pool(name="pvp", bufs=2, space="PSUM"))
        denp = ctx.enter_context(tc.tile_pool(name="denp", bufs=1, space="PSUM"))
        finp = ctx.enter_context(tc.tile_pool(name="finp", bufs=2, space="PSUM"))

        # ---- constants ----
        ones_col = const.tile([128, 128], f16, name="ones_col")
        nc.vector.memset(ones_col, 1.0)
        ones_row = const.tile([1, 128], f32, name="ones_row")
        nc.vector.memset(ones_row, 1.0)
        ind_fwd = const.tile([128, 4], f32, name="ind_fwd")   # [ch, grp] one-hot
        nc.vector.memset(ind_fwd, 0.0)
        for g in range(4):
            nc.vector.memset(ind_fwd[g * GS:(g + 1) * GS, g:g + 1], 1.0)
        ind_bwd = const.tile([4, 128], f32, name="ind_bwd")   # [grp, ch] one-hot
        nc.vector.memset(ind_bwd[0:4, :], 0.0)
        for g in range(4):
            # engines can't start at partition g∉{0,32,64,96}; DMA can
            nc.sync.dma_start(out=ind_bwd[g:g + 1, g * GS:(g + 1) * GS],
                              in_=ones_row[0:1, 0:GS])
        eps_t = const.tile([128, 1], f32, name="eps_t")
        nc.vector.memset(eps_t, EPS)

        # ---- load x FIRST in [128,1024] chunks, split across two DMA
        #      engines (each dma_start costs ~600ns of issue time serially
        #      per engine) ----
        xq = [[None] * 4 for _ in range(2)]
        dma_engs = [nc.sync, nc.gpsimd, nc.scalar]
        for j in range(4):
            for ct in range(2):
                t = xqp.tile([128, 1024], f32, tag="xq", name=f"xq{ct}_{j}")
                eng = dma_engs[(j * 2 + ct) % 3]
                eng.dma_start(
                    out=t,
                    in_=x_kv[ct * 128:(ct + 1) * 128, j * 1024:(j + 1) * 1024])
                xq[ct][j] = t

        # ---- packed weights / biases (2 + 1 DMA issues) ----
        wpk = []
        for ci in range(2):
            w = const.tile([128, 4 * C], f32, name=f"wpk{ci}")
            nc.sync.dma_start(out=w, in_=w_pack[ci * 128:(ci + 1) * 128, :])
            wpk.append(w)
        vp = const.tile([128, 12], f32, name="vp")
        nc.gpsimd.dma_start(out=vp, in_=vec_pack[:, :])
        wt = [wpk[ci][:, 0:3 * C] for ci in range(2)]
        owt = [wpk[ci][:, 3 * C:4 * C] for ci in range(2)]
        qb = [vp[:, j:j + 1] for j in range(6)]
        nw = [vp[:, 6 + ci:7 + ci] for ci in range(2)]
        nb = [vp[:, 8 + ci:9 + ci] for ci in range(2)]
        ob = [vp[:, 10 + ci:11 + ci] for ci in range(2)]

        # fp16 casts of matmul weights (fp32 on PE = 2 HW passes at half rate)
        wt16, owt16 = [], []
        for ci in range(2):
            w16 = const.tile([128, 3 * C], f16, name=f"wt16_{ci}")
            nc.vector.tensor_copy(out=w16, in_=wt[ci])
            wt16.append(w16)
            o16 = const.tile([128, C], f16, name=f"owt16_{ci}")
            nc.vector.tensor_copy(out=o16, in_=owt[ci])
            owt16.append(o16)
        qb16v = []
        for j in (4, 5):
            t16 = const.tile([128, 1], f16, name=f"qb16_{j}")
            nc.vector.tensor_copy(out=t16, in_=qb[j])
            qb16v.append(t16)

        # ct0: DVE bn_stats (Welford); ct1: ACT accumulate sum & sumsq
        # per chunk -- the two engines run in parallel.
        stats0 = smal.tile([128, 8, 6], f32, tag="stats", name="stats0")
        for j in range(4):
            for s in range(2):
                nc.vector.bn_stats(
                    out=stats0[:, j * 2 + s, :],
                    in_=xq[0][j][:, s * 512:(s + 1) * 512])
        sc_scr = smal.tile([128, 1024], f32, tag="sc_scr", name="sc_scr")
        acc1 = smal.tile([128, 8], f32, tag="acc1", name="acc1")  # sums|sqsums
        for j in range(4):
            nc.scalar.activation(out=sc_scr, in_=xq[1][j], func=Act.Identity,
                                 accum_out=acc1[:, j:j + 1])
            nc.scalar.activation(out=sc_scr, in_=xq[1][j], func=Act.Square,
                                 accum_out=acc1[:, 4 + j:5 + j])

        # group stats -> per-channel affine (A, B)
        AB = []
        for ct in range(2):
            st2 = smal.tile([128, 2], f32, tag="st2", name=f"st2{ct}")
            if ct == 0:
                mv = smal.tile([128, 2], f32, tag="mv", name=f"mv{ct}")
                nc.vector.bn_aggr(out=mv, in_=stats0)
                nc.vector.tensor_copy(out=st2[:, 0:1], in_=mv[:, 0:1])
                nc.vector.tensor_mul(out=st2[:, 1:2], in0=mv[:, 0:1], in1=mv[:, 0:1])
                nc.vector.tensor_add(out=st2[:, 1:2], in0=st2[:, 1:2], in1=mv[:, 1:2])
            else:
                # st2 = [mean, E[x^2]] from the 4+4 partial sums
                pa = smal.tile([128, 4], f32, tag="pa", name="pa1")
                nc.vector.tensor_add(out=pa[:, 0:2], in0=acc1[:, 0:2],
                                     in1=acc1[:, 2:4])
                nc.vector.tensor_add(out=pa[:, 2:4], in0=acc1[:, 4:6],
                                     in1=acc1[:, 6:8])
                pb = smal.tile([128, 2], f32, tag="pb", name="pb1")
                nc.vector.tensor_add(out=pb[:, 0:1], in0=pa[:, 0:1], in1=pa[:, 1:2])
                nc.vector.tensor_add(out=pb[:, 1:2], in0=pa[:, 2:3], in1=pa[:, 3:4])
                nc.vector.tensor_scalar_mul(out=st2, in0=pb, scalar1=1.0 / N)
            gp = psp.tile([4, 2], f32, tag="ps", name=f"gp{ct}")
            nc.tensor.matmul(gp, ind_fwd, st2, start=True, stop=True)
            gsb = smal.tile([4, 2], f32, tag="gsb", name=f"gsb{ct}")
            nc.scalar.mul(out=gsb, in_=gp, mul=1.0 / GS)
            chp = psp.tile([128, 2], f32, tag="ps", name=f"chp{ct}")
            nc.tensor.matmul(chp, ind_bwd, gsb, start=True, stop=True)
            ch = smal.tile([128, 2], f32, tag="ch", name=f"ch{ct}")
            nc.vector.tensor_copy(out=ch, in_=chp)
            var = smal.tile([128, 1], f32, tag="var", name=f"var{ct}")
            nc.vector.tensor_mul(out=var, in0=ch[:, 0:1], in1=ch[:, 0:1])
            nc.vector.tensor_sub(out=var, in0=ch[:, 1:2], in1=var)
            std = smal.tile([128, 1], f32, tag="std", name=f"std{ct}")
            nc.scalar.activation(out=std, in_=var, func=Act.Sqrt, bias=eps_t)
            rstd = smal.tile([128, 1], f32, tag="rstd", name=f"rstd{ct}")
            nc.vector.reciprocal(out=rstd, in_=std)
            A = smal.tile([128, 1], f32, tag="A", name=f"A{ct}")
            nc.vector.tensor_mul(out=A, in0=nw[ct], in1=rstd)
            Bb = smal.tile([128, 1], f32, tag="B", name=f"B{ct}")
            nc.vector.tensor_mul(out=Bb, in0=ch[:, 0:1], in1=A)
            nc.vector.tensor_sub(out=Bb, in0=nb[ct], in1=Bb)
            AB.append((A, Bb))

        # apply affine -> fp16 h chunks (ct0 on DVE, ct1 on ACT, in parallel)
        h16 = [[None] * 4 for _ in range(2)]
        for j in range(4):
            for ct in range(2):
                ht = hp.tile([128, 1024], f16, tag="h16", name=f"h16_{ct}_{j}")
                if ct == 0:
                    nc.vector.tensor_scalar(
                        out=ht, in0=xq[ct][j],
                        scalar1=AB[ct][0], scalar2=AB[ct][1],
                        op0=Alu.mult, op1=Alu.add)
                else:
                    nc.scalar.activation(
                        out=ht, in_=xq[ct][j], func=Act.Identity,
                        bias=AB[ct][1], scale=AB[ct][0])
                h16[ct][j] = ht

        def hsl(ct, nch):  # [128,512] slice of h16 for 512-col chunk nch
            return h16[ct][nch // 2][:, (nch % 2) * 512:((nch % 2) + 1) * 512]

        def hmt(ct, mt):   # [128,128] slice of h16 for 128-col tile mt
            return h16[ct][mt // 8][:, (mt % 8) * 128:((mt % 8) + 1) * 128]

        # ---- q conv (queries = first NQ cols) ----
        q_sb = [qp.tile([128, NQ], f16, tag="q", name=f"q{co}") for co in range(2)]
        for co in range(2):
            for nch in range(NQ // 512):
                pq = psp.tile([128, 512], f32, tag="ps", name=f"pq{co}_{nch}")
                nc.tensor.matmul(pq, wt16[0][:, co * 128:(co + 1) * 128],
                                 hsl(0, nch), start=True, stop=False)
                nc.tensor.matmul(pq, wt16[1][:, co * 128:(co + 1) * 128],
                                 hsl(1, nch), start=False, stop=True)
                nc.vector.tensor_scalar_add(
                    out=q_sb[co][:, nch * 512:(nch + 1) * 512],
                    in0=pq, scalar1=qb[co])

        # ---- k conv ----
        k_sb = [kp.tile([128, N], f16, tag="k", name=f"k{co}") for co in range(2)]
        for co in range(2):
            for nch in range(N // 512):
                pk = psp.tile([128, 512], f32, tag="ps", name=f"pk{co}_{nch}")
                nc.tensor.matmul(pk, wt16[0][:, 256 + co * 128:256 + (co + 1) * 128],
                                 hsl(0, nch), start=True, stop=False)
                nc.tensor.matmul(pk, wt16[1][:, 256 + co * 128:256 + (co + 1) * 128],
                                 hsl(1, nch), start=False, stop=True)
                nc.vector.tensor_scalar_add(
                    out=k_sb[co][:, nch * 512:(nch + 1) * 512],
                    in0=pk, scalar1=qb[2 + co])

        # ---- vT conv (no bias; folded into effb) ----
        vt_sb = []
        for mt in range(32):
            pvt = psp.tile([128, 256], f32, tag="ps", name=f"pvt{mt}")
            nc.tensor.matmul(pvt, hmt(0, mt), wt16[0][:, 512:768],
                             start=True, stop=False)
            nc.tensor.matmul(pvt, hmt(1, mt), wt16[1][:, 512:768],
                             start=False, stop=True)
            vt = vtp.tile([128, 256], f16, tag="vt", name=f"vt{mt}")
            nc.vector.tensor_copy(out=vt, in_=pvt)
            vt_sb.append(vt)

        # ---- effective output bias: out_w @ v_bias + out_b ----
        effb = []
        for co in range(2):
            pe_ = psp.tile([128, 1], f32, tag="ps", name=f"peffb{co}")
            nc.tensor.matmul(pe_, owt16[0][:, co * 128:(co + 1) * 128], qb16v[0],
                             start=True, stop=False)
            nc.tensor.matmul(pe_, owt16[1][:, co * 128:(co + 1) * 128], qb16v[1],
                             start=False, stop=True)
            e = smal.tile([128, 1], f32, tag="effb", name=f"effb{co}")
            nc.vector.tensor_add(out=e, in0=pe_, in1=ob[co])
            effb.append(e)

        # ---- attention ----
        def st_chunk(t, mch):
            ps_s = psp.tile([128, 512], f32, tag="ps", name=f"s_{t}_{mch}")
            nc.tensor.matmul(ps_s, k_sb[0][:, mch * 128:(mch + 1) * 128],
                             q_sb[0][:, t * 512:(t + 1) * 512],
                             start=True, stop=False)
            nc.tensor.matmul(ps_s, k_sb[1][:, mch * 128:(mch + 1) * 128],
                             q_sb[1][:, t * 512:(t + 1) * 512],
                             start=False, stop=True)
            es = esp.tile([128, 512], f16, tag="es", name=f"es_{t}_{mch}")
            nc.scalar.activation(out=es, in_=ps_s, func=Act.Exp, scale=SCALE)
            return es

        def epilogue(t, pv, den):
            # den arrives already broadcast across partitions ([128,512]);
            # 1/den is entirely off the PE critical path (normalization
            # commutes through the output projection as a per-query scale)
            dsb = sb2.tile([128, 512], f32, tag="dsb", name=f"dsb_{t}")
            nc.vector.tensor_copy(out=dsb, in_=den)
            rdb = sb2.tile([128, 512], f32, tag="rdb", name=f"rdb_{t}")
            nc.vector.reciprocal(out=rdb, in_=dsb)
            pvsb = []
            for co in range(2):
                p = sb2.tile([128, 512], f16, tag=f"pvsb{co}",
                             name=f"pvsb_{t}_{co}")
                nc.scalar.copy(out=p, in_=pv[co])  # frees the pv psum bank
                pvsb.append(p)
            for co in range(2):
                fin = finp.tile([128, 512], f32, tag="fin", name=f"fin_{t}_{co}")
                nc.tensor.matmul(fin, owt16[0][:, co * 128:(co + 1) * 128],
                                 pvsb[0], start=True, stop=False)
                nc.tensor.matmul(fin, owt16[1][:, co * 128:(co + 1) * 128],
                                 pvsb[1], start=False, stop=True)
                rsb = sb2.tile([128, 512], f32, tag=f"rsb{co}", name=f"r_{t}_{co}")
                nc.sync.dma_start(
                    out=rsb,
                    in_=x_kv[co * 128:(co + 1) * 128, t * 512:(t + 1) * 512])
                t1 = sb2.tile([128, 512], f32, tag=f"t1{co}", name=f"t1_{t}_{co}")
                nc.vector.tensor_mul(out=t1, in0=fin, in1=rdb)
                ysb = sb2.tile([128, 512], f32, tag=f"ysb{co}", name=f"y_{t}_{co}")
                nc.vector.scalar_tensor_tensor(
                    out=ysb, in0=t1, scalar=effb[co], in1=rsb,
                    op0=Alu.add, op1=Alu.add)
                nc.sync.dma_start(
                    out=y[co * 128:(co + 1) * 128, t * 512:(t + 1) * 512],
                    in_=ysb)

        NT = NQ // 512
        for t in range(NT):
            pv = [pvp.tile([128, 512], f32, tag="pv", name=f"pv_{t}_{co}")
                  for co in range(2)]
            den = denp.tile([128, 512], f32, tag="den", name=f"den_{t}")
            es_q = {0: st_chunk(t, 0), 1: st_chunk(t, 1)}
            for mch in range(32):
                es = es_q.pop(mch)
                st, sp = (mch == 0), (mch == 31)
                nc.tensor.matmul(pv[0], vt_sb[mch][:, 0:128], es, start=st, stop=sp)
                nc.tensor.matmul(pv[1], vt_sb[mch][:, 128:256], es, start=st, stop=sp)
                nc.tensor.matmul(den, ones_col, es, start=st, stop=sp)
                if mch + 2 < 32:
                    es_q[mch + 2] = st_chunk(t, mch + 2)
            epilogue(t, pv, den)

    _split_multiwait_instructions(nc)
    return nc


def _get_nc():
    if "nc" not in _CACHE:
        _CACHE["nc"] = _build()
    return _CACHE["nc"]


def _prep_in_maps(x, norm_w, norm_b, qkv_w, qkv_b, out_w, out_b):
    xr = np.ascontiguousarray(np.asarray(x, dtype=np.float32).reshape(B, C, N))
    wT = np.asarray(qkv_w, np.float32).T
    owT = np.asarray(out_w, np.float32).T
    w_pack = np.ascontiguousarray(np.concatenate([wT, owT], axis=1))
    qb1 = np.asarray(qkv_b, np.float32)
    vec_pack = np.ascontiguousarray(np.stack(
        [qb1[0:128], qb1[128:256], qb1[256:384], qb1[384:512],
         qb1[512:640], qb1[640:768],
         np.asarray(norm_w, np.float32)[0:128],
         np.asarray(norm_w, np.float32)[128:256],
         np.asarray(norm_b, np.float32)[0:128],
         np.asarray(norm_b, np.float32)[128:256],
         np.asarray(out_b, np.float32)[0:128],
         np.asarray(out_b, np.float32)[128:256]], axis=1))

    in_maps = []
    for c in range(8):
        b, h = divmod(c, 2)
        off = h * NQ
        xb = xr[b]
        if off:
            xroll = np.ascontiguousarray(
                np.concatenate([xb[:, off:], xb[:, :off]], axis=1))
        else:
            xroll = xb
        in_maps.append({"x_kv": xroll, "w_pack": w_pack, "vec_pack": vec_pack})
    return in_maps


def kernel(x, norm_w, norm_b, qkv_w, qkv_b, out_w, out_b):
    from concourse.bass_utils import run_bass_kernel_spmd

    nc = _get_nc()
    in_maps = _prep_in_maps(x, norm_w, norm_b, qkv_w, qkv_b, out_w, out_b)
    res = run_bass_kernel_spmd(nc, in_maps, core_ids=list(range(8)))
    out = np.empty((B, C, N), np.float32)
    for c in range(8):
        b, h = divmod(c, 2)
        off = h * NQ
        out[b][:, off:off + NQ] = res.results[c]["y"]
    return out.reshape(B, C, 16, 16, 16)
